# revision 20
# baseline (speedup 1.0000x reference)
"""Trainium2 Bass kernel for nn_BasicRecurrentEntityEncoder.

Full-input contract: kernel(**inputs) takes the complete (unsharded) numpy
inputs and returns the full [B, K, D] float32 output. Internally the batch
is sharded over 8 NeuronCores (data parallel, no collectives), the embedding
bag-of-words gather runs through dma_gather against a per-core compacted
bf16 table, and the 64-step entity recurrence runs in a transposed
[D, (b,k)] layout with bf16 matmul operands.

Key device-side structure per core (B_local=16, K=32, D=256, S=64):
  - 8 gather groups of 128 sentences (4096 tokens, 1 dma_gather each);
    word-sum via block-ones matmuls into PSUM; TensorE transpose to build
    E^T [256, 1024] incrementally.
  - precompute  kVT = V^T keys^T,  eW = W^T E^T,  GK = E^T^T keys^T.
  - scan step: PSUM accumulates U^T h + kVT + eW_bcast via matmuls;
    gate logits PSUM = E_t^T h + GK; sigmoid = 1/(1+exp(-x)) (exp on
    ScalarE, reciprocal_approx_fast on VectorE); normalization
    rsqrt = exp(-0.5*ln(ss+eps)) on ScalarE -- every ScalarE function
    lives in the natural_log_exp_and_others activation table so no
    table reloads occur.
  - mask folding: h_new = normalize(h + (m*gate) .* h_tilda) is exact for
    masked rows because h is always 0 or unit-norm.
"""

import sys

if "/opt/trn_rl_repo" not in sys.path:
    sys.path.insert(0, "/opt/trn_rl_repo")

import numpy as np
import ml_dtypes

from concourse import bacc, mybir
import concourse.bass as bass
import concourse.tile as tile
from concourse.bass_utils import run_bass_kernel_spmd
from concourse.masks import make_identity

# Force every ScalarE activation onto the one table set that covers all the
# functions this kernel uses (relu/square/exp/ln/copy/identity). The default
# chooser greedily picks the first set per function (exp -> set 0,
# ln -> set 5), inserting a ~550ns table reload per Ln/Exp pair on the
# critical path. Padding the dict keeps act_func_set_id indices aligned
# with act_info.json while making only the all-covering set usable.
_ONE_SET = "natural_log_exp_and_others"


import concourse.hw_specs as _hw_specs
_ORIG_TABLES = _hw_specs.get_activation_tables


def _patched_tables(module_arch):
    real = _ORIG_TABLES(module_arch)
    names = list(real.keys())
    assert _ONE_SET in names, names
    out = {}
    for n in names:
        if n == _ONE_SET:
            out[n] = real[n]
            break
        out[n] = set()
    return out


def _install_table_patch():
    import functools
    cached = functools.cache(_patched_tables)
    bacc.get_activation_tables = cached
    _hw_specs.get_activation_tables = cached


_install_table_patch()

F32 = mybir.dt.float32
BF16 = mybir.dt.bfloat16
I16 = mybir.dt.int16
AF = mybir.ActivationFunctionType
OP = mybir.AluOpType

B, S, L, K, D = 128, 64, 32, 32, 256
NC = 8
BL = B // NC              # 16 batch rows per core
BK = BL * K               # 512 = free dim of the state
NG = 8                    # gather groups per core (128 sentences each)
TOKG = 128 * L            # 4096 tokens per group
TABLE_ROWS = 32768        # compacted per-core vocab (unique ids <= 32768)
EPS = 1e-12

_CACHED = {}


def _build_program():
    nc = bacc.Bacc("TRN2", target_bir_lowering=False, debug=False, num_devices=NC)

    table = nc.dram_tensor("table", [TABLE_ROWS, D], BF16, kind="ExternalInput").ap()
    idx16 = nc.dram_tensor("idx16", [128, NG * TOKG // 16], I16, kind="ExternalInput").ap()
    keysT = nc.dram_tensor("keysT", [D, BK], BF16, kind="ExternalInput").ap()
    Umat = nc.dram_tensor("Umat", [D, D], BF16, kind="ExternalInput").ap()
    Vmat = nc.dram_tensor("Vmat", [D, D], BF16, kind="ExternalInput").ap()
    Wmat = nc.dram_tensor("Wmat", [D, D], BF16, kind="ExternalInput").ap()
    mrow = nc.dram_tensor("mrow", [8, 2 * S], F32, kind="ExternalInput").ap()
    bdm = nc.dram_tensor("bdm", [BL, BK], BF16, kind="ExternalInput").ap()
    hout = nc.dram_tensor("hout", [BK, D], F32, kind="ExternalOutput").ap()

    with tile.TileContext(nc) as tc:
        _emit(nc, tc, table, idx16, keysT, Umat, Vmat, Wmat, mrow, bdm, hout)
    nc.compile()
    return nc


def _emit(nc, tc, table, idx16, keysT, Umat, Vmat, Wmat, mrow, bdm, hout):
    from contextlib import ExitStack

    ctx = ExitStack()
    const = ctx.enter_context(tc.tile_pool(name="const", bufs=1))
    persist = ctx.enter_context(tc.tile_pool(name="persist", bufs=1))
    gpool = ctx.enter_context(tc.tile_pool(name="g", bufs=2))
    work = ctx.enter_context(tc.tile_pool(name="work", bufs=3))
    hpool = ctx.enter_context(tc.tile_pool(name="h", bufs=2))
    # PSUM budget: 8 banks total. psh0+psh1 + {ps, pst, psg, psgb, pss, psi} = 8.
    psH = ctx.enter_context(tc.tile_pool(name="psH", bufs=1, space="PSUM"))
    psS = ctx.enter_context(tc.tile_pool(name="psS", bufs=1, space="PSUM"))

    # ---- constants into SBUF ----
    sb_idx = const.tile([128, NG * TOKG // 16], I16)
    nc.sync.dma_start(out=sb_idx[:], in_=idx16[:])
    kT = [const.tile([128, BK], BF16, tag=f"kT{j}", name=f"kT{j}") for j in range(2)]
    for j in range(2):
        nc.sync.dma_start(out=kT[j][:], in_=keysT[128 * j:128 * (j + 1), :])
    sbU = [const.tile([128, D], BF16, tag=f"sbU{j}", name=f"sbU{j}") for j in range(2)]
    sbV = [const.tile([128, D], BF16, tag=f"sbV{j}", name=f"sbV{j}") for j in range(2)]
    sbW = [const.tile([128, D], BF16, tag=f"sbW{j}", name=f"sbW{j}") for j in range(2)]
    for j in range(2):
        nc.sync.dma_start(out=sbU[j][:], in_=Umat[128 * j:128 * (j + 1), :])
        nc.sync.dma_start(out=sbV[j][:], in_=Vmat[128 * j:128 * (j + 1), :])
        nc.sync.dma_start(out=sbW[j][:], in_=Wmat[128 * j:128 * (j + 1), :])
    sb_m = const.tile([8, 2 * S], F32)
    nc.sync.dma_start(out=sb_m[:], in_=mrow[:])
    sb_bd = const.tile([BL, BK], BF16)
    nc.sync.dma_start(out=sb_bd[:], in_=bdm[:])

    I128 = const.tile([128, 128], BF16)
    make_identity(nc, I128[:])
    ones8 = const.tile([8, 128], BF16)
    nc.vector.memset(ones8[:], 1.0)
    ones128 = const.tile([128, 1], BF16)
    nc.vector.memset(ones128[:], 1.0)
    ones1 = const.tile([1, 128], BF16)
    nc.vector.memset(ones1[:], 1.0)
    epsap = const.tile([1, 1], F32)
    nc.vector.memset(epsap[:], EPS)
    # word-sum reducers: Ablk[i][p, m] = 1 iff m == 4*i + p//32.
    # Slot c contributes sentences 4c+q; accumulating 16 slots with
    # patterns i = c%16 fills a 64-sentence PSUM block (base 0 or 64).
    Ablk = []
    for i in range(16):
        a = const.tile([128, 64], BF16, tag=f"Ablk{i}", name=f"Ablk{i}")
        nc.vector.memset(a[:], 0.0)
        for q in range(4):
            nc.vector.memset(a[32 * q:32 * (q + 1), 4 * i + q:4 * i + q + 1], 1.0)
        Ablk.append(a)

    # ---- persistent intermediates ----
    ET = [persist.tile([128, NG * 128], BF16, tag=f"ET{j}", name=f"ET{j}") for j in range(2)]   # E^T  [d, (g,ds,b)]
    eW = [persist.tile([128, NG * 128], BF16, tag=f"eWt{j}", name=f"eWt{j}") for j in range(2)]   # W^T E^T
    kVT = [persist.tile([128, BK], BF16, tag=f"kVT{j}", name=f"kVT{j}") for j in range(2)]        # V^T keys^T

    # kVT = V^T @ keysT   (out[de, bk] = sum_d V[d,de] keysT[d,bk])
    for m in range(2):
        ps = psS.tile([128, BK], F32, tag="psm0", name="pskv")
        nc.tensor.matmul(ps[:], lhsT=sbV[0][:, 128 * m:128 * (m + 1)], rhs=kT[0][:],
                         start=True, stop=False)
        nc.tensor.matmul(ps[:], lhsT=sbV[1][:, 128 * m:128 * (m + 1)], rhs=kT[1][:],
                         start=False, stop=True)
        nc.vector.tensor_copy(out=kVT[m][:], in_=ps[:])

    # ---- gather groups ----
    for g in range(NG):
        G = gpool.tile([128, L, D], BF16, tag="G")
        nc.gpsimd.dma_gather(
            out_ap=G[:], in_ap=table[:],
            idxs_ap=sb_idx[:, (TOKG // 16) * g:(TOKG // 16) * (g + 1)],
            num_idxs=TOKG, num_idxs_reg=TOKG, elem_size=D, single_packet=False,
        )
        # word-sum: slot c holds words of sentences 4c..4c+3; accumulate
        # 8 slots per 32-aligned PSUM block.
        psE = psS.tile([128, D], F32, tag="psm0", name="psE")
        for c in range(L):
            j, i = c // 16, c % 16
            nc.tensor.matmul(psE[64 * j:64 * (j + 1), :], lhsT=Ablk[i][:],
                             rhs=G[:, c, :], start=(i == 0), stop=(i == 15))
        enc = work.tile([128, D], BF16, tag="enc")
        nc.scalar.copy(out=enc[:], in_=psE[:])
        # transpose -> ET columns for this group
        for j in range(2):
            pt = psS.tile([128, 128], BF16, tag="psm1", name="pt")
            nc.tensor.transpose(pt[:], enc[:, 128 * j:128 * (j + 1)], I128[:])
            nc.vector.tensor_copy(out=ET[j][:, 128 * g:128 * (g + 1)], in_=pt[:])
        # eW = W^T @ ET_g
        for m in range(2):
            pw = psS.tile([128, 128], F32, tag="psm1", name="pw")
            nc.tensor.matmul(pw[:], lhsT=sbW[0][:, 128 * m:128 * (m + 1)],
                             rhs=ET[0][:, 128 * g:128 * (g + 1)], start=True, stop=False)
            nc.tensor.matmul(pw[:], lhsT=sbW[1][:, 128 * m:128 * (m + 1)],
                             rhs=ET[1][:, 128 * g:128 * (g + 1)], start=False, stop=True)
            nc.vector.tensor_copy(out=eW[m][:, 128 * g:128 * (g + 1)], in_=pw[:])

    # ---- scan: two independent batch groups (b 0-7 | b 8-15) pipelined ----
    # Each group owns a 256-wide bk slice and its own PSUM banks, so the two
    # serial dependency chains interleave across engines. Within a group the
    # state h packs both de-tiles side by side ([d0-127 | d128-255] columns)
    # so elementwise V/S ops run full-width [128, 512] in single instructions;
    # the gate/inv broadcasts are duplicated across both column halves.
    HB = BK // 2  # 256
    h = [hpool.tile([128, BK], BF16, tag=f"h{gb}", name=f"h{gb}")
         for gb in range(2)]
    for gb in range(2):
        nc.vector.memset(h[gb][:], 0.0)

    for t in range(S):
        g, ds = t // 8, t % 8
        hn = [None, None]
        for gb in range(2):
            cg = 128 * g + 16 * ds + 8 * gb  # ET/eW cols for this step+group
            bks = slice(HB * gb, HB * (gb + 1))
            hg = h[gb]

            # pshG packs both de tiles: [:, 0:256] = de 0-127, [:, 256:512] = de 128-255
            pshG = psH.tile([128, BK], F32, tag=f"psh{gb}", name=f"psh{gb}")
            for m in range(2):
                msl = slice(HB * m, HB * (m + 1))
                nc.tensor.matmul(pshG[:, msl], lhsT=sbU[0][:, 128 * m:128 * (m + 1)],
                                 rhs=hg[:, 0:HB], start=True, stop=False)
                nc.tensor.matmul(pshG[:, msl], lhsT=sbU[1][:, 128 * m:128 * (m + 1)],
                                 rhs=hg[:, HB:BK], start=False, stop=False)
                nc.tensor.matmul(pshG[:, msl], lhsT=I128[:], rhs=kVT[m][:, bks],
                                 start=False, stop=False)
                ew_bc = eW[m][:, cg:cg + 8].unsqueeze(2).broadcast_to([128, 8, 32])
                nc.tensor.matmul(pshG[:, msl], lhsT=I128[:], rhs=ew_bc,
                                 start=False, stop=True)

            # psMISC: [0:8, 0:256] = gate logits, [0:1, 256:512] = sumsq
            psM = psS.tile([128, BK], F32, tag=f"psm{gb}", name=f"psm{gb}")
            psg = psM[0:8, 0:HB]
            nc.tensor.matmul(psg, lhsT=ET[0][:, cg:cg + 8], rhs=hg[:, 0:HB],
                             start=True, stop=False)
            nc.tensor.matmul(psg, lhsT=ET[1][:, cg:cg + 8], rhs=hg[:, HB:BK],
                             start=False, stop=False)
            nc.tensor.matmul(psg, lhsT=ET[0][:, cg:cg + 8], rhs=kT[0][:, bks],
                             start=False, stop=False)
            nc.tensor.matmul(psg, lhsT=ET[1][:, cg:cg + 8], rhs=kT[1][:, bks],
                             start=False, stop=True)

            # sigmoid = 1/(1+exp(-x)): exp+add on ScalarE, recip on VectorE.
            # No clamp: |logits| < ~30 for this model scale (exp(30) ~ 1e13,
            # far inside reciprocal_approx_fast's safe range).
            eg = work.tile([8, HB], F32, tag=f"eg{gb}", name=f"eg{gb}")
            nc.scalar.activation(eg[:], psg, AF.Exp, scale=-1.0)
            egp = work.tile([8, HB], F32, tag=f"egp{gb}", name=f"egp{gb}")
            nc.scalar.activation(egp[:], eg[:], AF.Identity, bias=1.0)
            sg = work.tile([8, HB], F32, tag=f"sg{gb}", name=f"sg{gb}")
            nc.vector.reciprocal_approx_fast(out=sg[:], in_=egp[:])
            gm = work.tile([8, HB], BF16, tag=f"gm{gb}", name=f"gm{gb}")
            nc.vector.scalar_tensor_tensor(
                out=gm[:], in0=sg[:], scalar=sb_m[0:8, 2 * t + gb:2 * t + gb + 1],
                in1=sb_bd[0:8, 0:HB], op0=OP.mult, op1=OP.mult)
            # gate broadcast duplicated into both column halves
            psBg = psS.tile([128, BK], F32, tag=f"psbg{gb}", name=f"psbg{gb}")
            nc.tensor.matmul(psBg[:, 0:HB], lhsT=ones8[:], rhs=gm[:],
                             start=True, stop=True)
            nc.tensor.matmul(psBg[:, HB:BK], lhsT=ones8[:], rhs=gm[:],
                             start=True, stop=True)

            # full-width elementwise: r = relu(psh); u = r*gate; upd = u + h
            r = work.tile([128, BK], BF16, tag=f"r{gb}", name=f"r{gb}")
            nc.scalar.activation(r[:], pshG[:], AF.Relu)
            u = work.tile([128, BK], BF16, tag=f"u{gb}", name=f"u{gb}")
            nc.vector.tensor_tensor(out=u[:], in0=r[:], in1=psBg[:], op=OP.mult)
            upd = work.tile([128, BK], BF16, tag=f"upd{gb}", name=f"upd{gb}")
            nc.vector.tensor_tensor(out=upd[:], in0=u[:], in1=hg[:], op=OP.add)
            sq = work.tile([128, BK], BF16, tag=f"sq{gb}", name=f"sq{gb}")
            nc.scalar.activation(sq[:], upd[:], AF.Square)

            pss = psM[0:1, HB:BK]
            nc.tensor.matmul(pss, lhsT=ones128[:], rhs=sq[:, 0:HB],
                             start=True, stop=False)
            nc.tensor.matmul(pss, lhsT=ones128[:], rhs=sq[:, HB:BK],
                             start=False, stop=True)
            lns = work.tile([1, HB], F32, tag=f"lns{gb}", name=f"lns{gb}")
            nc.scalar.activation(lns[:], pss, AF.Ln, bias=epsap[:])
            inv = work.tile([1, HB], BF16, tag=f"inv{gb}", name=f"inv{gb}")
            nc.scalar.activation(inv[:], lns[:], AF.Exp, scale=-0.5)
            psBi = psS.tile([128, BK], F32, tag=f"psbi{gb}", name=f"psbi{gb}")
            nc.tensor.matmul(psBi[:, 0:HB], lhsT=ones1[:], rhs=inv[:],
                             start=True, stop=True)
            nc.tensor.matmul(psBi[:, HB:BK], lhsT=ones1[:], rhs=inv[:],
                             start=True, stop=True)

            hn[gb] = hpool.tile([128, BK], BF16, tag=f"h{gb}", name=f"hn{gb}")
            nc.vector.tensor_tensor(out=hn[gb][:], in0=upd[:], in1=psBi[:],
                                    op=OP.mult)
        h = hn

    # ---- output: transpose h^T [256, 512] -> [512, 256] fp32 ----
    for q in range(4):
        gb, half = q // 2, q % 2
        ho = work.tile([128, D], F32, tag="ho")
        for j in range(2):
            pt = psS.tile([128, 128], BF16, tag="psm0", name="ptout")
            nc.tensor.transpose(pt[:], h[gb][:, HB * j + 128 * half:
                                             HB * j + 128 * half + 128], I128[:])
            nc.vector.tensor_copy(out=ho[:, 128 * j:128 * (j + 1)], in_=pt[:])
        nc.sync.dma_start(out=hout[128 * q:128 * (q + 1), :], in_=ho[:])

    ctx.close()


def _prep_core(pr, mask, keys_c, emb):
    """Host-side marshaling for one core's shard."""
    uniq, inv = np.unique(pr, return_inverse=True)
    assert len(uniq) <= TABLE_ROWS
    table = np.zeros((TABLE_ROWS, D), dtype=ml_dtypes.bfloat16)
    table[: len(uniq)] = emb[uniq].astype(ml_dtypes.bfloat16)
    ranks = inv.reshape(BL, S, L).astype(np.int16)

    # token order per group g: i = (ds*16 + b)*32 + w
    idx_groups = []
    for g in range(NG):
        blk = ranks[:, 8 * g:8 * (g + 1), :]          # [b, ds, w]
        lst = blk.transpose(1, 0, 2).reshape(-1)      # [(ds, b, w)] length 4096
        idx_groups.append(np.tile(lst.reshape(TOKG // 16, 16).T, (8, 1)))
    idx16 = np.concatenate(idx_groups, axis=1).astype(np.int16)  # [128, NG*256]

    keysT = np.ascontiguousarray(
        keys_c.reshape(BK, D).T).astype(ml_dtypes.bfloat16)      # [256, 512]
    # mrow2[j, 2t+gb] = mask[8*gb + j, t]  (two pipelined batch groups)
    m = mask.astype(np.float32)                                  # [16, 64]
    mrow2 = np.zeros((8, 2 * S), np.float32)
    for gb in range(2):
        mrow2[:, gb::2] = m[8 * gb:8 * (gb + 1), :]
    return table, idx16, keysT, mrow2


def kernel(prgrph, prgrph_mask, keys, embedding_matrix, U, V, W):
    prgrph = np.asarray(prgrph)
    prgrph_mask = np.asarray(prgrph_mask)
    keys = np.asarray(keys, dtype=np.float32)
    emb = np.asarray(embedding_matrix, dtype=np.float32)
    U = np.asarray(U, dtype=np.float32)
    V = np.asarray(V, dtype=np.float32)
    W = np.asarray(W, dtype=np.float32)

    if "nc" not in _CACHED:
        _CACHED["nc"] = _build_program()
    nc = _CACHED["nc"]

    bd = (np.arange(BL)[:, None] == (np.arange(BK)[None, :] // K)).astype(
        ml_dtypes.bfloat16)
    Ub, Vb, Wb = (x.astype(ml_dtypes.bfloat16) for x in (U, V, W))

    in_maps = []
    for c in range(NC):
        sl = slice(BL * c, BL * (c + 1))
        table, idx16, keysT, mrow = _prep_core(
            prgrph[sl], prgrph_mask[sl, :, 0], keys[sl], emb)
        in_maps.append({
            "table": table, "idx16": idx16, "keysT": keysT,
            "Umat": Ub, "Vmat": Vb, "Wmat": Wb,
            "mrow": mrow, "bdm": bd,
        })

    res = run_bass_kernel_spmd(nc, in_maps, core_ids=list(range(NC)))
    out = np.concatenate(
        [res.results[c]["hout"].reshape(BL, K, D) for c in range(NC)], axis=0)
    return out.astype(np.float32)


# revision 22
# speedup vs baseline: 1.0314x; 1.0314x over previous
"""Trainium2 Bass kernel for nn_BasicRecurrentEntityEncoder.

Full-input contract: kernel(**inputs) takes the complete (unsharded) numpy
inputs and returns the full [B, K, D] float32 output. Internally the batch
is sharded over 8 NeuronCores (data parallel, no collectives), the embedding
bag-of-words gather runs through dma_gather against a per-core compacted
bf16 table, and the 64-step entity recurrence runs in a transposed
[D, (b,k)] layout with bf16 matmul operands.

Key device-side structure per core (B_local=16, K=32, D=256, S=64):
  - 8 gather groups of 128 sentences (4096 tokens, 1 dma_gather each);
    word-sum via block-ones matmuls into PSUM; TensorE transpose to build
    E^T [256, 1024] incrementally.
  - precompute  kVT = V^T keys^T,  eW = W^T E^T,  GK = E^T^T keys^T.
  - scan step: PSUM accumulates U^T h + kVT + eW_bcast via matmuls;
    gate logits PSUM = E_t^T h + GK; sigmoid = 1/(1+exp(-x)) (exp on
    ScalarE, reciprocal_approx_fast on VectorE); normalization
    rsqrt = exp(-0.5*ln(ss+eps)) on ScalarE -- every ScalarE function
    lives in the natural_log_exp_and_others activation table so no
    table reloads occur.
  - mask folding: h_new = normalize(h + (m*gate) .* h_tilda) is exact for
    masked rows because h is always 0 or unit-norm.
"""

import sys

if "/opt/trn_rl_repo" not in sys.path:
    sys.path.insert(0, "/opt/trn_rl_repo")

import numpy as np
import ml_dtypes

from concourse import bacc, mybir
import concourse.bass as bass
import concourse.tile as tile
from concourse.bass_utils import run_bass_kernel_spmd
from concourse.masks import make_identity

# Force every ScalarE activation onto the one table set that covers all the
# functions this kernel uses (relu/square/exp/ln/copy/identity). The default
# chooser greedily picks the first set per function (exp -> set 0,
# ln -> set 5), inserting a ~550ns table reload per Ln/Exp pair on the
# critical path. Padding the dict keeps act_func_set_id indices aligned
# with act_info.json while making only the all-covering set usable.
_ONE_SET = "natural_log_exp_and_others"


import concourse.hw_specs as _hw_specs
_ORIG_TABLES = _hw_specs.get_activation_tables


def _patched_tables(module_arch):
    real = _ORIG_TABLES(module_arch)
    names = list(real.keys())
    assert _ONE_SET in names, names
    out = {}
    for n in names:
        if n == _ONE_SET:
            out[n] = real[n]
            break
        out[n] = set()
    return out


def _install_table_patch():
    import functools
    cached = functools.cache(_patched_tables)
    bacc.get_activation_tables = cached
    _hw_specs.get_activation_tables = cached


_install_table_patch()

# Custom DVE op: out ~= 1/(1 + in0) in ONE VectorE instruction (8 ALU
# stages): u = in0+1; seed y0 = bitcast(~bits(u)); t = u*y0 lands in
# [-4.5, -4] for any positive u; quadratic minimax fixup P(t) ~= 1/t gives
# out = y0*P(t) at ~1e-5 relative error. Replaces the separate ScalarE
# "+1" hop feeding reciprocal_approx_fast in the sigmoid.
import concourse.dve_ops as _dve_ops
from concourse.dve_spec import AluOp as _AluOp, Bin as _Bin, Spec as _Spec
from concourse.dve_spec import C0 as _C0, C1 as _C1, C2 as _C2, One as _One
from concourse.dve_spec import Src0 as _Src0, lower as _dve_lower
from concourse.dve_spec import _has_src1 as _dve_has_src1
from concourse.dve_uop import DveOpSpec as _DveOpSpec


def _fit_recip1p_consts():
    t = np.linspace(-4.5, -4.0, 2001)
    c = np.polyfit(t, 1.0 / t, 2)  # [c2, c1, c0]
    return float(c[2]), float(c[1]), float(c[0])


_R1P_C2, _R1P_C1, _R1P_C0 = (lambda c: (c[0], c[1], c[2]))(
    np.polyfit(np.linspace(-4.5, -4.0, 2001),
               1.0 / np.linspace(-4.5, -4.0, 2001), 2))


def _recip1p_ref(in0, in1, c0, c1, c2):
    u = (np.asarray(in0, np.float32) + np.float32(1.0)).astype(np.float32)
    y0 = (~u.view(np.int32)).view(np.float32)
    t = u * y0
    return y0 * (c0 + t * (c1 + c2 * t))


def _make_recip1p():
    u = _Bin(_AluOp.ADD, _Src0, _One)
    y0 = _Bin(_AluOp.BITWISE_NOT, u, u)
    t = u * y0
    spec = _Spec(body=y0 * (_C0 + t * (_C1 + _C2 * t)), reference=_recip1p_ref)
    name = "RECIP1P_APPROX_ANT"
    row = 1 + len(_dve_ops.OPS)
    assert row < 0x20
    shas = {}
    for ver in ("v3", "v4"):
        s = _DveOpSpec(name=name, opcode=row, uops=_dve_lower(spec, ver=ver),
                       rd1_en=_dve_has_src1(spec))
        shas[ver] = s.sha(ver)
    op = _dve_ops.DveOp(name, spec, subdim=False, uops_sha=shas)
    _dve_ops.OPS.append(op)
    _dve_ops._SUB_OPCODE_FOR_NAME[name] = row
    _dve_ops.CUSTOM_DVE_SPECS[name] = spec
    return op


_RECIP1P = _make_recip1p()

F32 = mybir.dt.float32
BF16 = mybir.dt.bfloat16
I16 = mybir.dt.int16
AF = mybir.ActivationFunctionType
OP = mybir.AluOpType

B, S, L, K, D = 128, 64, 32, 32, 256
NC = 8
BL = B // NC              # 16 batch rows per core
BK = BL * K               # 512 = free dim of the state
NG = 8                    # gather groups per core (128 sentences each)
TOKG = 128 * L            # 4096 tokens per group
TABLE_ROWS = 32768        # compacted per-core vocab (unique ids <= 32768)
EPS = 1e-12

_CACHED = {}


def _build_program():
    nc = bacc.Bacc("TRN2", target_bir_lowering=False, debug=False, num_devices=NC)

    table = nc.dram_tensor("table", [TABLE_ROWS, D], BF16, kind="ExternalInput").ap()
    idx16 = nc.dram_tensor("idx16", [128, NG * TOKG // 16], I16, kind="ExternalInput").ap()
    keysT = nc.dram_tensor("keysT", [D, BK], BF16, kind="ExternalInput").ap()
    Umat = nc.dram_tensor("Umat", [D, D], BF16, kind="ExternalInput").ap()
    Vmat = nc.dram_tensor("Vmat", [D, D], BF16, kind="ExternalInput").ap()
    Wmat = nc.dram_tensor("Wmat", [D, D], BF16, kind="ExternalInput").ap()
    mrow = nc.dram_tensor("mrow", [8, 2 * S], F32, kind="ExternalInput").ap()
    bdm = nc.dram_tensor("bdm", [BL, BK], BF16, kind="ExternalInput").ap()
    hout = nc.dram_tensor("hout", [BK, D], F32, kind="ExternalOutput").ap()

    with tile.TileContext(nc) as tc:
        _emit(nc, tc, table, idx16, keysT, Umat, Vmat, Wmat, mrow, bdm, hout)
    nc.compile()
    return nc


def _emit(nc, tc, table, idx16, keysT, Umat, Vmat, Wmat, mrow, bdm, hout):
    from contextlib import ExitStack

    ctx = ExitStack()
    const = ctx.enter_context(tc.tile_pool(name="const", bufs=1))
    persist = ctx.enter_context(tc.tile_pool(name="persist", bufs=1))
    gpool = ctx.enter_context(tc.tile_pool(name="g", bufs=2))
    work = ctx.enter_context(tc.tile_pool(name="work", bufs=3))
    hpool = ctx.enter_context(tc.tile_pool(name="h", bufs=2))
    # PSUM budget: 8 banks total. psh0+psh1 + {ps, pst, psg, psgb, pss, psi} = 8.
    psH = ctx.enter_context(tc.tile_pool(name="psH", bufs=1, space="PSUM"))
    psS = ctx.enter_context(tc.tile_pool(name="psS", bufs=1, space="PSUM"))

    # ---- constants into SBUF ----
    sb_idx = const.tile([128, NG * TOKG // 16], I16)
    nc.sync.dma_start(out=sb_idx[:], in_=idx16[:])
    kT = [const.tile([128, BK], BF16, tag=f"kT{j}", name=f"kT{j}") for j in range(2)]
    for j in range(2):
        nc.sync.dma_start(out=kT[j][:], in_=keysT[128 * j:128 * (j + 1), :])
    sbU = [const.tile([128, D], BF16, tag=f"sbU{j}", name=f"sbU{j}") for j in range(2)]
    sbV = [const.tile([128, D], BF16, tag=f"sbV{j}", name=f"sbV{j}") for j in range(2)]
    sbW = [const.tile([128, D], BF16, tag=f"sbW{j}", name=f"sbW{j}") for j in range(2)]
    for j in range(2):
        nc.sync.dma_start(out=sbU[j][:], in_=Umat[128 * j:128 * (j + 1), :])
        nc.sync.dma_start(out=sbV[j][:], in_=Vmat[128 * j:128 * (j + 1), :])
        nc.sync.dma_start(out=sbW[j][:], in_=Wmat[128 * j:128 * (j + 1), :])
    sb_m = const.tile([8, 2 * S], F32)
    nc.sync.dma_start(out=sb_m[:], in_=mrow[:])
    sb_bd = const.tile([BL, BK], BF16)
    nc.sync.dma_start(out=sb_bd[:], in_=bdm[:])

    I128 = const.tile([128, 128], BF16)
    make_identity(nc, I128[:])
    ones8 = const.tile([8, 128], BF16)
    nc.vector.memset(ones8[:], 1.0)
    ones128 = const.tile([128, 1], BF16)
    nc.vector.memset(ones128[:], 1.0)
    ones1 = const.tile([1, 128], BF16)
    nc.vector.memset(ones1[:], 1.0)
    epsap = const.tile([1, 1], F32)
    nc.vector.memset(epsap[:], EPS)
    # word-sum reducers: Ablk[i][p, m] = 1 iff m == 4*i + p//32.
    # Slot c contributes sentences 4c+q; accumulating 16 slots with
    # patterns i = c%16 fills a 64-sentence PSUM block (base 0 or 64).
    Ablk = []
    for i in range(16):
        a = const.tile([128, 64], BF16, tag=f"Ablk{i}", name=f"Ablk{i}")
        nc.vector.memset(a[:], 0.0)
        for q in range(4):
            nc.vector.memset(a[32 * q:32 * (q + 1), 4 * i + q:4 * i + q + 1], 1.0)
        Ablk.append(a)

    # ---- persistent intermediates ----
    ET = [persist.tile([128, NG * 128], BF16, tag=f"ET{j}", name=f"ET{j}") for j in range(2)]   # E^T  [d, (g,ds,b)]
    eW = [persist.tile([128, NG * 128], BF16, tag=f"eWt{j}", name=f"eWt{j}") for j in range(2)]   # W^T E^T
    kVT = [persist.tile([128, BK], BF16, tag=f"kVT{j}", name=f"kVT{j}") for j in range(2)]        # V^T keys^T

    # kVT = V^T @ keysT   (out[de, bk] = sum_d V[d,de] keysT[d,bk])
    for m in range(2):
        ps = psS.tile([128, BK], F32, tag="psm0", name="pskv")
        nc.tensor.matmul(ps[:], lhsT=sbV[0][:, 128 * m:128 * (m + 1)], rhs=kT[0][:],
                         start=True, stop=False)
        nc.tensor.matmul(ps[:], lhsT=sbV[1][:, 128 * m:128 * (m + 1)], rhs=kT[1][:],
                         start=False, stop=True)
        nc.vector.tensor_copy(out=kVT[m][:], in_=ps[:])

    # ---- gather groups ----
    for g in range(NG):
        G = gpool.tile([128, L, D], BF16, tag="G")
        nc.gpsimd.dma_gather(
            out_ap=G[:], in_ap=table[:],
            idxs_ap=sb_idx[:, (TOKG // 16) * g:(TOKG // 16) * (g + 1)],
            num_idxs=TOKG, num_idxs_reg=TOKG, elem_size=D, single_packet=False,
        )
        # word-sum: slot c holds words of sentences 4c..4c+3; accumulate
        # 8 slots per 32-aligned PSUM block.
        psE = psS.tile([128, D], F32, tag="psm0", name="psE")
        for c in range(L):
            j, i = c // 16, c % 16
            nc.tensor.matmul(psE[64 * j:64 * (j + 1), :], lhsT=Ablk[i][:],
                             rhs=G[:, c, :], start=(i == 0), stop=(i == 15))
        enc = work.tile([128, D], BF16, tag="enc")
        nc.scalar.copy(out=enc[:], in_=psE[:])
        # transpose -> ET columns for this group
        for j in range(2):
            pt = psS.tile([128, 128], BF16, tag="psm1", name="pt")
            nc.tensor.transpose(pt[:], enc[:, 128 * j:128 * (j + 1)], I128[:])
            nc.vector.tensor_copy(out=ET[j][:, 128 * g:128 * (g + 1)], in_=pt[:])
        # eW = W^T @ ET_g
        for m in range(2):
            pw = psS.tile([128, 128], F32, tag="psm1", name="pw")
            nc.tensor.matmul(pw[:], lhsT=sbW[0][:, 128 * m:128 * (m + 1)],
                             rhs=ET[0][:, 128 * g:128 * (g + 1)], start=True, stop=False)
            nc.tensor.matmul(pw[:], lhsT=sbW[1][:, 128 * m:128 * (m + 1)],
                             rhs=ET[1][:, 128 * g:128 * (g + 1)], start=False, stop=True)
            nc.vector.tensor_copy(out=eW[m][:, 128 * g:128 * (g + 1)], in_=pw[:])

    # ---- scan: two independent batch groups (b 0-7 | b 8-15) pipelined ----
    # Each group owns a 256-wide bk slice and its own PSUM banks, so the two
    # serial dependency chains interleave across engines. Within a group the
    # state h packs both de-tiles side by side ([d0-127 | d128-255] columns)
    # so elementwise V/S ops run full-width [128, 512] in single instructions;
    # the gate/inv broadcasts are duplicated across both column halves.
    HB = BK // 2  # 256
    h = [hpool.tile([128, BK], BF16, tag=f"h{gb}", name=f"h{gb}")
         for gb in range(2)]
    for gb in range(2):
        nc.vector.memset(h[gb][:], 0.0)

    for t in range(S):
        g, ds = t // 8, t % 8
        hn = [None, None]
        for gb in range(2):
            cg = 128 * g + 16 * ds + 8 * gb  # ET/eW cols for this step+group
            bks = slice(HB * gb, HB * (gb + 1))
            hg = h[gb]

            # pshG packs both de tiles: [:, 0:256] = de 0-127, [:, 256:512] = de 128-255
            pshG = psH.tile([128, BK], F32, tag=f"psh{gb}", name=f"psh{gb}")
            for m in range(2):
                msl = slice(HB * m, HB * (m + 1))
                nc.tensor.matmul(pshG[:, msl], lhsT=sbU[0][:, 128 * m:128 * (m + 1)],
                                 rhs=hg[:, 0:HB], start=True, stop=False)
                nc.tensor.matmul(pshG[:, msl], lhsT=sbU[1][:, 128 * m:128 * (m + 1)],
                                 rhs=hg[:, HB:BK], start=False, stop=False)
                nc.tensor.matmul(pshG[:, msl], lhsT=I128[:], rhs=kVT[m][:, bks],
                                 start=False, stop=False)
                ew_bc = eW[m][:, cg:cg + 8].unsqueeze(2).broadcast_to([128, 8, 32])
                nc.tensor.matmul(pshG[:, msl], lhsT=I128[:], rhs=ew_bc,
                                 start=False, stop=True)

            # psMISC: [0:8, 0:256] = gate logits, [0:1, 256:512] = sumsq
            psM = psS.tile([128, BK], F32, tag=f"psm{gb}", name=f"psm{gb}")
            psg = psM[0:8, 0:HB]
            nc.tensor.matmul(psg, lhsT=ET[0][:, cg:cg + 8], rhs=hg[:, 0:HB],
                             start=True, stop=False)
            nc.tensor.matmul(psg, lhsT=ET[1][:, cg:cg + 8], rhs=hg[:, HB:BK],
                             start=False, stop=False)
            nc.tensor.matmul(psg, lhsT=ET[0][:, cg:cg + 8], rhs=kT[0][:, bks],
                             start=False, stop=False)
            nc.tensor.matmul(psg, lhsT=ET[1][:, cg:cg + 8], rhs=kT[1][:, bks],
                             start=False, stop=True)

            # sigmoid = 1/(1+exp(-x)): exp+add on ScalarE, recip on VectorE.
            # No clamp: |logits| < ~30 for this model scale (exp(30) ~ 1e13,
            # far inside reciprocal_approx_fast's safe range).
            eg = work.tile([8, HB], F32, tag=f"eg{gb}", name=f"eg{gb}")
            nc.scalar.activation(eg[:], psg, AF.Exp, scale=-1.0)
            sg = work.tile([8, HB], F32, tag=f"sg{gb}", name=f"sg{gb}")
            nc.vector._custom_dve(_RECIP1P, out=sg[:], in0=eg[:],
                                  s0=float(_R1P_C0), s1=float(_R1P_C1),
                                  imm2=float(_R1P_C2))
            gm = work.tile([8, HB], BF16, tag=f"gm{gb}", name=f"gm{gb}")
            nc.vector.scalar_tensor_tensor(
                out=gm[:], in0=sg[:], scalar=sb_m[0:8, 2 * t + gb:2 * t + gb + 1],
                in1=sb_bd[0:8, 0:HB], op0=OP.mult, op1=OP.mult)
            # gate broadcast duplicated into both column halves
            psBg = psS.tile([128, BK], F32, tag=f"psbg{gb}", name=f"psbg{gb}")
            nc.tensor.matmul(psBg[:, 0:HB], lhsT=ones8[:], rhs=gm[:],
                             start=True, stop=True)
            nc.tensor.matmul(psBg[:, HB:BK], lhsT=ones8[:], rhs=gm[:],
                             start=True, stop=True)

            # full-width elementwise: r = relu(psh); u = r*gate; upd = u + h
            r = work.tile([128, BK], BF16, tag=f"r{gb}", name=f"r{gb}")
            nc.scalar.activation(r[:], pshG[:], AF.Relu)
            u = work.tile([128, BK], BF16, tag=f"u{gb}", name=f"u{gb}")
            nc.vector.tensor_tensor(out=u[:], in0=r[:], in1=psBg[:], op=OP.mult)
            upd = work.tile([128, BK], BF16, tag=f"upd{gb}", name=f"upd{gb}")
            nc.vector.tensor_tensor(out=upd[:], in0=u[:], in1=hg[:], op=OP.add)
            sq = work.tile([128, BK], BF16, tag=f"sq{gb}", name=f"sq{gb}")
            nc.vector.tensor_tensor(out=sq[:], in0=upd[:], in1=upd[:], op=OP.mult)

            pss = psM[0:1, HB:BK]
            nc.tensor.matmul(pss, lhsT=ones128[:], rhs=sq[:, 0:HB],
                             start=True, stop=False)
            nc.tensor.matmul(pss, lhsT=ones128[:], rhs=sq[:, HB:BK],
                             start=False, stop=True)
            lns = work.tile([1, HB], F32, tag=f"lns{gb}", name=f"lns{gb}")
            nc.scalar.activation(lns[:], pss, AF.Ln, bias=epsap[:])
            inv = work.tile([1, HB], BF16, tag=f"inv{gb}", name=f"inv{gb}")
            nc.scalar.activation(inv[:], lns[:], AF.Exp, scale=-0.5)
            psBi = psS.tile([128, BK], F32, tag=f"psbi{gb}", name=f"psbi{gb}")
            nc.tensor.matmul(psBi[:, 0:HB], lhsT=ones1[:], rhs=inv[:],
                             start=True, stop=True)
            nc.tensor.matmul(psBi[:, HB:BK], lhsT=ones1[:], rhs=inv[:],
                             start=True, stop=True)

            hn[gb] = hpool.tile([128, BK], BF16, tag=f"h{gb}", name=f"hn{gb}")
            nc.vector.tensor_tensor(out=hn[gb][:, 0:HB], in0=upd[:, 0:HB],
                                    in1=psBi[:, 0:HB], op=OP.mult)
            nc.vector.tensor_tensor(out=hn[gb][:, HB:BK], in0=upd[:, HB:BK],
                                    in1=psBi[:, HB:BK], op=OP.mult)
        h = hn

    # ---- output: transpose h^T [256, 512] -> [512, 256] fp32 ----
    for q in range(4):
        gb, half = q // 2, q % 2
        ho = work.tile([128, D], F32, tag="ho")
        for j in range(2):
            pt = psS.tile([128, 128], BF16, tag="psm0", name="ptout")
            nc.tensor.transpose(pt[:], h[gb][:, HB * j + 128 * half:
                                             HB * j + 128 * half + 128], I128[:])
            nc.vector.tensor_copy(out=ho[:, 128 * j:128 * (j + 1)], in_=pt[:])
        nc.sync.dma_start(out=hout[128 * q:128 * (q + 1), :], in_=ho[:])

    ctx.close()


def _prep_core(pr, mask, keys_c, emb):
    """Host-side marshaling for one core's shard."""
    uniq, inv = np.unique(pr, return_inverse=True)
    assert len(uniq) <= TABLE_ROWS
    table = np.zeros((TABLE_ROWS, D), dtype=ml_dtypes.bfloat16)
    table[: len(uniq)] = emb[uniq].astype(ml_dtypes.bfloat16)
    ranks = inv.reshape(BL, S, L).astype(np.int16)

    # token order per group g: i = (ds*16 + b)*32 + w
    idx_groups = []
    for g in range(NG):
        blk = ranks[:, 8 * g:8 * (g + 1), :]          # [b, ds, w]
        lst = blk.transpose(1, 0, 2).reshape(-1)      # [(ds, b, w)] length 4096
        idx_groups.append(np.tile(lst.reshape(TOKG // 16, 16).T, (8, 1)))
    idx16 = np.concatenate(idx_groups, axis=1).astype(np.int16)  # [128, NG*256]

    keysT = np.ascontiguousarray(
        keys_c.reshape(BK, D).T).astype(ml_dtypes.bfloat16)      # [256, 512]
    # mrow2[j, 2t+gb] = mask[8*gb + j, t]  (two pipelined batch groups)
    m = mask.astype(np.float32)                                  # [16, 64]
    mrow2 = np.zeros((8, 2 * S), np.float32)
    for gb in range(2):
        mrow2[:, gb::2] = m[8 * gb:8 * (gb + 1), :]
    return table, idx16, keysT, mrow2


def kernel(prgrph, prgrph_mask, keys, embedding_matrix, U, V, W):
    prgrph = np.asarray(prgrph)
    prgrph_mask = np.asarray(prgrph_mask)
    keys = np.asarray(keys, dtype=np.float32)
    emb = np.asarray(embedding_matrix, dtype=np.float32)
    U = np.asarray(U, dtype=np.float32)
    V = np.asarray(V, dtype=np.float32)
    W = np.asarray(W, dtype=np.float32)

    if "nc" not in _CACHED:
        _CACHED["nc"] = _build_program()
    nc = _CACHED["nc"]

    bd = (np.arange(BL)[:, None] == (np.arange(BK)[None, :] // K)).astype(
        ml_dtypes.bfloat16)
    Ub, Vb, Wb = (x.astype(ml_dtypes.bfloat16) for x in (U, V, W))

    in_maps = []
    for c in range(NC):
        sl = slice(BL * c, BL * (c + 1))
        table, idx16, keysT, mrow = _prep_core(
            prgrph[sl], prgrph_mask[sl, :, 0], keys[sl], emb)
        in_maps.append({
            "table": table, "idx16": idx16, "keysT": keysT,
            "Umat": Ub, "Vmat": Vb, "Wmat": Wb,
            "mrow": mrow, "bdm": bd,
        })

    res = run_bass_kernel_spmd(nc, in_maps, core_ids=list(range(NC)))
    out = np.concatenate(
        [res.results[c]["hout"].reshape(BL, K, D) for c in range(NC)], axis=0)
    return out.astype(np.float32)


# revision 24
# speedup vs baseline: 5304.5578x; 5143.1452x over previous
"""Trainium2 Bass kernel for nn_BasicRecurrentEntityEncoder.

Full-input contract: kernel(**inputs) takes the complete (unsharded) numpy
inputs and returns the full [B, K, D] float32 output. Internally the batch
is sharded over 8 NeuronCores (data parallel, no collectives), the embedding
bag-of-words gather runs through dma_gather against a per-core compacted
bf16 table, and the 64-step entity recurrence runs in a transposed
[D, (b,k)] layout with bf16 matmul operands.

Key device-side structure per core (B_local=16, K=32, D=256, S=64):
  - 8 gather groups of 128 sentences (4096 tokens, 1 dma_gather each);
    word-sum via block-ones matmuls into PSUM; TensorE transpose to build
    E^T [256, 1024] incrementally.
  - precompute  kVT = V^T keys^T  and  eW = W^T E^T  once per group.
  - the scan runs as TWO independent batch groups (b 0-7 | b 8-15), each
    with its own PSUM banks, so their serial dependency chains pipeline
    across engines. Per step and group: PSUM accumulates
    U^T h + kVT + eW_bcast via matmuls (identity / stride-0-broadcast
    rhs tricks); gate logits PSUM = E_t^T (h + keys); sigmoid =
    1/(1+exp(-x)) with exp on ScalarE and a one-instruction custom DVE
    op for 1/(1+y); normalization rsqrt = exp(-0.5*ln(ss+eps)) on
    ScalarE. Every ScalarE function lives in the
    natural_log_exp_and_others activation table so no table reloads
    occur (the default greedy chooser is patched out).
  - mask folding: h_new = normalize(h + (m*gate) .* h_tilda) is exact for
    masked rows because h is always 0 or unit-norm.
"""

import sys

if "/opt/trn_rl_repo" not in sys.path:
    sys.path.insert(0, "/opt/trn_rl_repo")

import numpy as np
import ml_dtypes

from concourse import bacc, mybir
import concourse.bass as bass
import concourse.tile as tile
from concourse.bass_utils import run_bass_kernel_spmd
from concourse.masks import make_identity

# Force every ScalarE activation onto the one table set that covers all the
# functions this kernel uses (relu/square/exp/ln/copy/identity). The default
# chooser greedily picks the first set per function (exp -> set 0,
# ln -> set 5), inserting a ~550ns table reload per Ln/Exp pair on the
# critical path. Padding the dict keeps act_func_set_id indices aligned
# with act_info.json while making only the all-covering set usable.
_ONE_SET = "natural_log_exp_and_others"


import concourse.hw_specs as _hw_specs
_ORIG_TABLES = _hw_specs.get_activation_tables


def _patched_tables(module_arch):
    real = _ORIG_TABLES(module_arch)
    names = list(real.keys())
    assert _ONE_SET in names, names
    out = {}
    for n in names:
        if n == _ONE_SET:
            out[n] = real[n]
            break
        out[n] = set()
    return out


def _install_table_patch():
    import functools
    cached = functools.cache(_patched_tables)
    bacc.get_activation_tables = cached
    _hw_specs.get_activation_tables = cached


_install_table_patch()

# Custom DVE op: out ~= 1/(1 + in0) in ONE VectorE instruction (8 ALU
# stages): u = in0+1; seed y0 = bitcast(~bits(u)); t = u*y0 lands in
# [-4.5, -4] for any positive u; quadratic minimax fixup P(t) ~= 1/t gives
# out = y0*P(t) at ~1e-5 relative error. Replaces the separate ScalarE
# "+1" hop feeding reciprocal_approx_fast in the sigmoid.
import concourse.dve_ops as _dve_ops
from concourse.dve_spec import AluOp as _AluOp, Bin as _Bin, Spec as _Spec
from concourse.dve_spec import C0 as _C0, C1 as _C1, C2 as _C2, One as _One
from concourse.dve_spec import Src0 as _Src0, lower as _dve_lower
from concourse.dve_spec import _has_src1 as _dve_has_src1
from concourse.dve_uop import DveOpSpec as _DveOpSpec


def _fit_recip1p_consts():
    t = np.linspace(-4.5, -4.0, 2001)
    c = np.polyfit(t, 1.0 / t, 2)  # [c2, c1, c0]
    return float(c[2]), float(c[1]), float(c[0])


_R1P_C2, _R1P_C1, _R1P_C0 = (lambda c: (c[0], c[1], c[2]))(
    np.polyfit(np.linspace(-4.5, -4.0, 2001),
               1.0 / np.linspace(-4.5, -4.0, 2001), 2))


def _recip1p_ref(in0, in1, c0, c1, c2):
    u = (np.asarray(in0, np.float32) + np.float32(1.0)).astype(np.float32)
    y0 = (~u.view(np.int32)).view(np.float32)
    t = u * y0
    return y0 * (c0 + t * (c1 + c2 * t))


def _make_recip1p():
    u = _Bin(_AluOp.ADD, _Src0, _One)
    y0 = _Bin(_AluOp.BITWISE_NOT, u, u)
    t = u * y0
    spec = _Spec(body=y0 * (_C0 + t * (_C1 + _C2 * t)), reference=_recip1p_ref)
    name = "RECIP1P_APPROX_ANT"
    row = 1 + len(_dve_ops.OPS)
    assert row < 0x20
    shas = {}
    for ver in ("v3", "v4"):
        s = _DveOpSpec(name=name, opcode=row, uops=_dve_lower(spec, ver=ver),
                       rd1_en=_dve_has_src1(spec))
        shas[ver] = s.sha(ver)
    op = _dve_ops.DveOp(name, spec, subdim=False, uops_sha=shas)
    _dve_ops.OPS.append(op)
    _dve_ops._SUB_OPCODE_FOR_NAME[name] = row
    _dve_ops.CUSTOM_DVE_SPECS[name] = spec
    return op


_RECIP1P = _make_recip1p()

F32 = mybir.dt.float32
BF16 = mybir.dt.bfloat16
I16 = mybir.dt.int16
AF = mybir.ActivationFunctionType
OP = mybir.AluOpType

B, S, L, K, D = 128, 64, 32, 32, 256
NC = 8
BL = B // NC              # 16 batch rows per core
BK = BL * K               # 512 = free dim of the state
NG = 8                    # gather groups per core (128 sentences each)
TOKG = 128 * L            # 4096 tokens per group
TABLE_ROWS = 32768        # compacted per-core vocab (unique ids <= 32768)
EPS = 1e-12

_CACHED = {}


def _build_program():
    nc = bacc.Bacc("TRN2", target_bir_lowering=False, debug=False, num_devices=NC)

    table = nc.dram_tensor("table", [TABLE_ROWS, D], BF16, kind="ExternalInput").ap()
    idx16 = nc.dram_tensor("idx16", [128, NG * TOKG // 16], I16, kind="ExternalInput").ap()
    keysT = nc.dram_tensor("keysT", [D, BK], BF16, kind="ExternalInput").ap()
    Umat = nc.dram_tensor("Umat", [D, D], BF16, kind="ExternalInput").ap()
    Vmat = nc.dram_tensor("Vmat", [D, D], BF16, kind="ExternalInput").ap()
    Wmat = nc.dram_tensor("Wmat", [D, D], BF16, kind="ExternalInput").ap()
    mrow = nc.dram_tensor("mrow", [8, 2 * S], F32, kind="ExternalInput").ap()
    bdm = nc.dram_tensor("bdm", [BL, BK], BF16, kind="ExternalInput").ap()
    hout = nc.dram_tensor("hout", [BK, D], F32, kind="ExternalOutput").ap()

    with tile.TileContext(nc) as tc:
        _emit(nc, tc, table, idx16, keysT, Umat, Vmat, Wmat, mrow, bdm, hout)
    nc.compile()
    return nc


def _emit(nc, tc, table, idx16, keysT, Umat, Vmat, Wmat, mrow, bdm, hout):
    from contextlib import ExitStack

    ctx = ExitStack()
    const = ctx.enter_context(tc.tile_pool(name="const", bufs=1))
    persist = ctx.enter_context(tc.tile_pool(name="persist", bufs=1))
    gpool = ctx.enter_context(tc.tile_pool(name="g", bufs=2))
    work = ctx.enter_context(tc.tile_pool(name="work", bufs=4))
    hpool = ctx.enter_context(tc.tile_pool(name="h", bufs=3))
    # PSUM budget: 8 banks total. psh0+psh1 + {ps, pst, psg, psgb, pss, psi} = 8.
    psH = ctx.enter_context(tc.tile_pool(name="psH", bufs=1, space="PSUM"))
    psS = ctx.enter_context(tc.tile_pool(name="psS", bufs=1, space="PSUM"))

    # ---- constants into SBUF ----
    sb_idx = const.tile([128, NG * TOKG // 16], I16)
    nc.sync.dma_start(out=sb_idx[:], in_=idx16[:])
    kT = [const.tile([128, BK], BF16, tag=f"kT{j}", name=f"kT{j}") for j in range(2)]
    for j in range(2):
        nc.sync.dma_start(out=kT[j][:], in_=keysT[128 * j:128 * (j + 1), :])
    sbU = [const.tile([128, D], BF16, tag=f"sbU{j}", name=f"sbU{j}") for j in range(2)]
    sbV = [const.tile([128, D], BF16, tag=f"sbV{j}", name=f"sbV{j}") for j in range(2)]
    sbW = [const.tile([128, D], BF16, tag=f"sbW{j}", name=f"sbW{j}") for j in range(2)]
    for j in range(2):
        nc.sync.dma_start(out=sbU[j][:], in_=Umat[128 * j:128 * (j + 1), :])
        nc.sync.dma_start(out=sbV[j][:], in_=Vmat[128 * j:128 * (j + 1), :])
        nc.sync.dma_start(out=sbW[j][:], in_=Wmat[128 * j:128 * (j + 1), :])
    sb_m = const.tile([8, 2 * S], F32)
    nc.sync.dma_start(out=sb_m[:], in_=mrow[:])
    sb_bd = const.tile([BL, BK], BF16)
    nc.sync.dma_start(out=sb_bd[:], in_=bdm[:])

    I128 = const.tile([128, 128], BF16)
    make_identity(nc, I128[:])
    ones8 = const.tile([8, 128], BF16)
    nc.vector.memset(ones8[:], 1.0)
    ones128 = const.tile([128, 1], BF16)
    nc.vector.memset(ones128[:], 1.0)
    ones1 = const.tile([1, 128], BF16)
    nc.vector.memset(ones1[:], 1.0)
    epsap = const.tile([1, 1], F32)
    nc.vector.memset(epsap[:], EPS)
    # word-sum reducers: Ablk[i][p, m] = 1 iff m == 4*i + p//32.
    # Slot c contributes sentences 4c+q; accumulating 16 slots with
    # patterns i = c%16 fills a 64-sentence PSUM block (base 0 or 64).
    Ablk = []
    for i in range(16):
        a = const.tile([128, 64], BF16, tag=f"Ablk{i}", name=f"Ablk{i}")
        nc.vector.memset(a[:], 0.0)
        for q in range(4):
            nc.vector.memset(a[32 * q:32 * (q + 1), 4 * i + q:4 * i + q + 1], 1.0)
        Ablk.append(a)

    # ---- persistent intermediates ----
    ET = [persist.tile([128, NG * 128], BF16, tag=f"ET{j}", name=f"ET{j}") for j in range(2)]   # E^T  [d, (g,ds,b)]
    eW = [persist.tile([128, NG * 128], BF16, tag=f"eWt{j}", name=f"eWt{j}") for j in range(2)]   # W^T E^T
    kVT = [persist.tile([128, BK], BF16, tag=f"kVT{j}", name=f"kVT{j}") for j in range(2)]        # V^T keys^T

    # kVT = V^T @ keysT   (out[de, bk] = sum_d V[d,de] keysT[d,bk])
    for m in range(2):
        ps = psS.tile([128, BK], F32, tag="psm0", name="pskv")
        nc.tensor.matmul(ps[:], lhsT=sbV[0][:, 128 * m:128 * (m + 1)], rhs=kT[0][:],
                         start=True, stop=False)
        nc.tensor.matmul(ps[:], lhsT=sbV[1][:, 128 * m:128 * (m + 1)], rhs=kT[1][:],
                         start=False, stop=True)
        nc.vector.tensor_copy(out=kVT[m][:], in_=ps[:])

    # ---- gather groups ----
    for g in range(NG):
        G = gpool.tile([128, L, D], BF16, tag="G")
        nc.gpsimd.dma_gather(
            out_ap=G[:], in_ap=table[:],
            idxs_ap=sb_idx[:, (TOKG // 16) * g:(TOKG // 16) * (g + 1)],
            num_idxs=TOKG, num_idxs_reg=TOKG, elem_size=D, single_packet=False,
        )
        # word-sum: slot c holds words of sentences 4c..4c+3; accumulate
        # 8 slots per 32-aligned PSUM block.
        psE = psS.tile([128, D], F32, tag="psm0", name="psE")
        for c in range(L):
            j, i = c // 16, c % 16
            nc.tensor.matmul(psE[64 * j:64 * (j + 1), :], lhsT=Ablk[i][:],
                             rhs=G[:, c, :], start=(i == 0), stop=(i == 15))
        enc = work.tile([128, D], BF16, tag="enc")
        nc.scalar.copy(out=enc[:], in_=psE[:])
        # transpose -> ET columns for this group
        for j in range(2):
            pt = psS.tile([128, 128], BF16, tag="psm1", name="pt")
            nc.tensor.transpose(pt[:], enc[:, 128 * j:128 * (j + 1)], I128[:])
            nc.vector.tensor_copy(out=ET[j][:, 128 * g:128 * (g + 1)], in_=pt[:])
        # eW = W^T @ ET_g
        for m in range(2):
            pw = psS.tile([128, 128], F32, tag="psm1", name="pw")
            nc.tensor.matmul(pw[:], lhsT=sbW[0][:, 128 * m:128 * (m + 1)],
                             rhs=ET[0][:, 128 * g:128 * (g + 1)], start=True, stop=False)
            nc.tensor.matmul(pw[:], lhsT=sbW[1][:, 128 * m:128 * (m + 1)],
                             rhs=ET[1][:, 128 * g:128 * (g + 1)], start=False, stop=True)
            nc.vector.tensor_copy(out=eW[m][:, 128 * g:128 * (g + 1)], in_=pw[:])

    # ---- scan: two independent batch groups (b 0-7 | b 8-15) pipelined ----
    # Each group owns a 256-wide bk slice and its own PSUM banks, so the two
    # serial dependency chains interleave across engines. Within a group the
    # state h packs both de-tiles side by side ([d0-127 | d128-255] columns)
    # so elementwise V/S ops run full-width [128, 512] in single instructions;
    # the gate/inv broadcasts are duplicated across both column halves.
    HB = BK // 2  # 256
    h = [hpool.tile([128, BK], BF16, tag=f"h{gb}", name=f"h{gb}")
         for gb in range(2)]
    for gb in range(2):
        nc.vector.memset(h[gb][:], 0.0)

    for t in range(S):
        g, ds = t // 8, t % 8
        hn = [None, None]
        for gb in range(2):
            cg = 128 * g + 16 * ds + 8 * gb  # ET/eW cols for this step+group
            bks = slice(HB * gb, HB * (gb + 1))
            hg = h[gb]

            # pshG packs both de tiles: [:, 0:256] = de 0-127, [:, 256:512] = de 128-255
            pshG = psH.tile([128, BK], F32, tag=f"psh{gb}", name=f"psh{gb}")
            for m in range(2):
                msl = slice(HB * m, HB * (m + 1))
                nc.tensor.matmul(pshG[:, msl], lhsT=sbU[0][:, 128 * m:128 * (m + 1)],
                                 rhs=hg[:, 0:HB], start=True, stop=False)
                nc.tensor.matmul(pshG[:, msl], lhsT=sbU[1][:, 128 * m:128 * (m + 1)],
                                 rhs=hg[:, HB:BK], start=False, stop=False)
                nc.tensor.matmul(pshG[:, msl], lhsT=I128[:], rhs=kVT[m][:, bks],
                                 start=False, stop=False)
                ew_bc = eW[m][:, cg:cg + 8].unsqueeze(2).broadcast_to([128, 8, 32])
                nc.tensor.matmul(pshG[:, msl], lhsT=I128[:], rhs=ew_bc,
                                 start=False, stop=True)

            # psMISC: [0:8, 0:256] = gate logits, [0:1, 256:512] = sumsq
            psM = psS.tile([128, BK], F32, tag=f"psm{gb}", name=f"psm{gb}")
            psg = psM[0:8, 0:HB]
            nc.tensor.matmul(psg, lhsT=ET[0][:, cg:cg + 8], rhs=hg[:, 0:HB],
                             start=True, stop=False)
            nc.tensor.matmul(psg, lhsT=ET[1][:, cg:cg + 8], rhs=hg[:, HB:BK],
                             start=False, stop=False)
            nc.tensor.matmul(psg, lhsT=ET[0][:, cg:cg + 8], rhs=kT[0][:, bks],
                             start=False, stop=False)
            nc.tensor.matmul(psg, lhsT=ET[1][:, cg:cg + 8], rhs=kT[1][:, bks],
                             start=False, stop=True)

            # sigmoid = 1/(1+exp(-x)): exp+add on ScalarE, recip on VectorE.
            # No clamp: |logits| < ~30 for this model scale (exp(30) ~ 1e13,
            # far inside reciprocal_approx_fast's safe range).
            eg = work.tile([8, HB], F32, tag=f"eg{gb}", name=f"eg{gb}")
            nc.scalar.activation(eg[:], psg, AF.Exp, scale=-1.0)
            sg = work.tile([8, HB], F32, tag=f"sg{gb}", name=f"sg{gb}")
            nc.vector._custom_dve(_RECIP1P, out=sg[:], in0=eg[:],
                                  s0=float(_R1P_C0), s1=float(_R1P_C1),
                                  imm2=float(_R1P_C2))
            gm = work.tile([8, HB], BF16, tag=f"gm{gb}", name=f"gm{gb}")
            nc.vector.scalar_tensor_tensor(
                out=gm[:], in0=sg[:], scalar=sb_m[0:8, 2 * t + gb:2 * t + gb + 1],
                in1=sb_bd[0:8, 0:HB], op0=OP.mult, op1=OP.mult)
            # gate broadcast duplicated into both column halves
            psBg = psS.tile([128, BK], F32, tag=f"psbg{gb}", name=f"psbg{gb}")
            nc.tensor.matmul(psBg[:, 0:HB], lhsT=ones8[:], rhs=gm[:],
                             start=True, stop=True)
            nc.tensor.matmul(psBg[:, HB:BK], lhsT=ones8[:], rhs=gm[:],
                             start=True, stop=True)

            # full-width elementwise: r = relu(psh); u = r*gate; upd = u + h
            r = work.tile([128, BK], BF16, tag=f"r{gb}", name=f"r{gb}")
            nc.scalar.activation(r[:], pshG[:], AF.Relu)
            u = work.tile([128, BK], BF16, tag=f"u{gb}", name=f"u{gb}")
            nc.vector.tensor_tensor(out=u[:], in0=r[:], in1=psBg[:], op=OP.mult)
            upd = work.tile([128, BK], BF16, tag=f"upd{gb}", name=f"upd{gb}")
            nc.vector.tensor_tensor(out=upd[:], in0=u[:], in1=hg[:], op=OP.add)
            sq = work.tile([128, BK], BF16, tag=f"sq{gb}", name=f"sq{gb}")
            nc.vector.tensor_tensor(out=sq[:], in0=upd[:], in1=upd[:], op=OP.mult)

            pss = psM[0:1, HB:BK]
            nc.tensor.matmul(pss, lhsT=ones128[:], rhs=sq[:, 0:HB],
                             start=True, stop=False)
            nc.tensor.matmul(pss, lhsT=ones128[:], rhs=sq[:, HB:BK],
                             start=False, stop=True)
            lns = work.tile([1, HB], F32, tag=f"lns{gb}", name=f"lns{gb}")
            nc.scalar.activation(lns[:], pss, AF.Ln, bias=epsap[:])
            inv = work.tile([1, HB], BF16, tag=f"inv{gb}", name=f"inv{gb}")
            nc.scalar.activation(inv[:], lns[:], AF.Exp, scale=-0.5)
            psBi = psS.tile([128, BK], F32, tag=f"psbi{gb}", name=f"psbi{gb}")
            nc.tensor.matmul(psBi[:, 0:HB], lhsT=ones1[:], rhs=inv[:],
                             start=True, stop=True)
            nc.tensor.matmul(psBi[:, HB:BK], lhsT=ones1[:], rhs=inv[:],
                             start=True, stop=True)

            hn[gb] = hpool.tile([128, BK], BF16, tag=f"h{gb}", name=f"hn{gb}")
            nc.vector.tensor_tensor(out=hn[gb][:, 0:HB], in0=upd[:, 0:HB],
                                    in1=psBi[:, 0:HB], op=OP.mult)
            nc.vector.tensor_tensor(out=hn[gb][:, HB:BK], in0=upd[:, HB:BK],
                                    in1=psBi[:, HB:BK], op=OP.mult)
        h = hn

    # ---- output: transpose h^T [256, 512] -> [512, 256] fp32 ----
    for q in range(4):
        gb, half = q // 2, q % 2
        ho = work.tile([128, D], F32, tag="ho")
        for j in range(2):
            pt = psS.tile([128, 128], BF16, tag="psm0", name="ptout")
            nc.tensor.transpose(pt[:], h[gb][:, HB * j + 128 * half:
                                             HB * j + 128 * half + 128], I128[:])
            nc.vector.tensor_copy(out=ho[:, 128 * j:128 * (j + 1)], in_=pt[:])
        nc.sync.dma_start(out=hout[128 * q:128 * (q + 1), :], in_=ho[:])

    ctx.close()


def _prep_core(pr, mask, keys_c, emb):
    """Host-side marshaling for one core's shard."""
    uniq, inv = np.unique(pr, return_inverse=True)
    assert len(uniq) <= TABLE_ROWS
    table = np.zeros((TABLE_ROWS, D), dtype=ml_dtypes.bfloat16)
    table[: len(uniq)] = emb[uniq].astype(ml_dtypes.bfloat16)
    ranks = inv.reshape(BL, S, L).astype(np.int16)

    # token order per group g: i = (ds*16 + b)*32 + w
    idx_groups = []
    for g in range(NG):
        blk = ranks[:, 8 * g:8 * (g + 1), :]          # [b, ds, w]
        lst = blk.transpose(1, 0, 2).reshape(-1)      # [(ds, b, w)] length 4096
        idx_groups.append(np.tile(lst.reshape(TOKG // 16, 16).T, (8, 1)))
    idx16 = np.concatenate(idx_groups, axis=1).astype(np.int16)  # [128, NG*256]

    keysT = np.ascontiguousarray(
        keys_c.reshape(BK, D).T).astype(ml_dtypes.bfloat16)      # [256, 512]
    # mrow2[j, 2t+gb] = mask[8*gb + j, t]  (two pipelined batch groups)
    m = mask.astype(np.float32)                                  # [16, 64]
    mrow2 = np.zeros((8, 2 * S), np.float32)
    for gb in range(2):
        mrow2[:, gb::2] = m[8 * gb:8 * (gb + 1), :]
    return table, idx16, keysT, mrow2


def kernel(prgrph, prgrph_mask, keys, embedding_matrix, U, V, W):
    prgrph = np.asarray(prgrph)
    prgrph_mask = np.asarray(prgrph_mask)
    keys = np.asarray(keys, dtype=np.float32)
    emb = np.asarray(embedding_matrix, dtype=np.float32)
    U = np.asarray(U, dtype=np.float32)
    V = np.asarray(V, dtype=np.float32)
    W = np.asarray(W, dtype=np.float32)

    if "nc" not in _CACHED:
        _CACHED["nc"] = _build_program()
    nc = _CACHED["nc"]

    bd = (np.arange(BL)[:, None] == (np.arange(BK)[None, :] // K)).astype(
        ml_dtypes.bfloat16)
    Ub, Vb, Wb = (x.astype(ml_dtypes.bfloat16) for x in (U, V, W))

    in_maps = []
    for c in range(NC):
        sl = slice(BL * c, BL * (c + 1))
        table, idx16, keysT, mrow = _prep_core(
            prgrph[sl], prgrph_mask[sl, :, 0], keys[sl], emb)
        in_maps.append({
            "table": table, "idx16": idx16, "keysT": keysT,
            "Umat": Ub, "Vmat": Vb, "Wmat": Wb,
            "mrow": mrow, "bdm": bd,
        })

    res = run_bass_kernel_spmd(nc, in_maps, core_ids=list(range(NC)))
    out = np.concatenate(
        [res.results[c]["hout"].reshape(BL, K, D) for c in range(NC)], axis=0)
    return out.astype(np.float32)


# revision 26
# speedup vs baseline: 5360.0058x; 1.0105x over previous
"""Trainium2 Bass kernel for nn_BasicRecurrentEntityEncoder.

Full-input contract: kernel(**inputs) takes the complete (unsharded) numpy
inputs and returns the full [B, K, D] float32 output. Internally the batch
is sharded over 8 NeuronCores (data parallel, no collectives), the embedding
bag-of-words gather runs through dma_gather against a per-core compacted
bf16 table, and the 64-step entity recurrence runs in a transposed
[D, (b,k)] layout with bf16 matmul operands.

Key device-side structure per core (B_local=16, K=32, D=256, S=64):
  - 8 gather groups of 128 sentences (4096 tokens, 1 dma_gather each);
    word-sum via block-ones matmuls into PSUM; TensorE transpose to build
    E^T [256, 1024] incrementally.
  - precompute  kVT = V^T keys^T  and  eW = W^T E^T  once per group.
  - the scan runs as TWO independent batch groups (b 0-7 | b 8-15), each
    with its own PSUM banks, so their serial dependency chains pipeline
    across engines. Per step and group: PSUM accumulates
    U^T h + kVT + eW_bcast via matmuls (identity / stride-0-broadcast
    rhs tricks); gate logits PSUM = E_t^T (h + keys); sigmoid =
    1/(1+exp(-x)) with exp on ScalarE and a one-instruction custom DVE
    op for 1/(1+y); normalization rsqrt = exp(-0.5*ln(ss+eps)) on
    ScalarE. Every ScalarE function lives in the
    natural_log_exp_and_others activation table so no table reloads
    occur (the default greedy chooser is patched out).
  - mask folding: h_new = normalize(h + (m*gate) .* h_tilda) is exact for
    masked rows because h is always 0 or unit-norm.
"""

import sys

if "/opt/trn_rl_repo" not in sys.path:
    sys.path.insert(0, "/opt/trn_rl_repo")

import numpy as np
import ml_dtypes

from concourse import bacc, mybir
import concourse.bass as bass
import concourse.tile as tile
from concourse.bass_utils import run_bass_kernel_spmd
from concourse.masks import make_identity

# Force every ScalarE activation onto the one table set that covers all the
# functions this kernel uses (relu/square/exp/ln/copy/identity). The default
# chooser greedily picks the first set per function (exp -> set 0,
# ln -> set 5), inserting a ~550ns table reload per Ln/Exp pair on the
# critical path. Padding the dict keeps act_func_set_id indices aligned
# with act_info.json while making only the all-covering set usable.
_ONE_SET = "natural_log_exp_and_others"


import concourse.hw_specs as _hw_specs
_ORIG_TABLES = _hw_specs.get_activation_tables


def _patched_tables(module_arch):
    real = _ORIG_TABLES(module_arch)
    names = list(real.keys())
    assert _ONE_SET in names, names
    out = {}
    for n in names:
        if n == _ONE_SET:
            out[n] = real[n]
            break
        out[n] = set()
    return out


def _install_table_patch():
    import functools
    cached = functools.cache(_patched_tables)
    bacc.get_activation_tables = cached
    _hw_specs.get_activation_tables = cached


_install_table_patch()

# Custom DVE op: out ~= 1/(1 + in0) in ONE VectorE instruction (8 ALU
# stages): u = in0+1; seed y0 = bitcast(~bits(u)); t = u*y0 lands in
# [-4.5, -4] for any positive u; quadratic minimax fixup P(t) ~= 1/t gives
# out = y0*P(t) at ~1e-5 relative error. Replaces the separate ScalarE
# "+1" hop feeding reciprocal_approx_fast in the sigmoid.
import concourse.dve_ops as _dve_ops
from concourse.dve_spec import AluOp as _AluOp, Bin as _Bin, Spec as _Spec
from concourse.dve_spec import C0 as _C0, C1 as _C1, C2 as _C2, One as _One
from concourse.dve_spec import Src0 as _Src0, lower as _dve_lower
from concourse.dve_spec import _has_src1 as _dve_has_src1
from concourse.dve_uop import DveOpSpec as _DveOpSpec


def _fit_recip1p_consts():
    t = np.linspace(-4.5, -4.0, 2001)
    c = np.polyfit(t, 1.0 / t, 2)  # [c2, c1, c0]
    return float(c[2]), float(c[1]), float(c[0])


_R1P_C2, _R1P_C1, _R1P_C0 = (lambda c: (c[0], c[1], c[2]))(
    np.polyfit(np.linspace(-4.5, -4.0, 2001),
               1.0 / np.linspace(-4.5, -4.0, 2001), 2))


def _recip1p_ref(in0, in1, c0, c1, c2):
    u = (np.asarray(in0, np.float32) + np.float32(1.0)).astype(np.float32)
    y0 = (~u.view(np.int32)).view(np.float32)
    t = u * y0
    return y0 * (c0 + t * (c1 + c2 * t))


def _make_recip1p():
    u = _Bin(_AluOp.ADD, _Src0, _One)
    y0 = _Bin(_AluOp.BITWISE_NOT, u, u)
    t = u * y0
    spec = _Spec(body=y0 * (_C0 + t * (_C1 + _C2 * t)), reference=_recip1p_ref)
    name = "RECIP1P_APPROX_ANT"
    row = 1 + len(_dve_ops.OPS)
    assert row < 0x20
    shas = {}
    for ver in ("v3", "v4"):
        s = _DveOpSpec(name=name, opcode=row, uops=_dve_lower(spec, ver=ver),
                       rd1_en=_dve_has_src1(spec))
        shas[ver] = s.sha(ver)
    op = _dve_ops.DveOp(name, spec, subdim=False, uops_sha=shas)
    _dve_ops.OPS.append(op)
    _dve_ops._SUB_OPCODE_FOR_NAME[name] = row
    _dve_ops.CUSTOM_DVE_SPECS[name] = spec
    return op


_RECIP1P = _make_recip1p()

F32 = mybir.dt.float32
BF16 = mybir.dt.bfloat16
I16 = mybir.dt.int16
AF = mybir.ActivationFunctionType
OP = mybir.AluOpType

B, S, L, K, D = 128, 64, 32, 32, 256
NC = 8
BL = B // NC              # 16 batch rows per core
BK = BL * K               # 512 = free dim of the state
NG = 8                    # gather groups per core (128 sentences each)
TOKG = 128 * L            # 4096 tokens per group
TABLE_ROWS = 32768        # compacted per-core vocab (unique ids <= 32768)
EPS = 1e-12

_CACHED = {}


def _build_program():
    nc = bacc.Bacc("TRN2", target_bir_lowering=False, debug=False, num_devices=NC)

    table = nc.dram_tensor("table", [TABLE_ROWS, D], BF16, kind="ExternalInput").ap()
    idx16 = nc.dram_tensor("idx16", [128, NG * TOKG // 16], I16, kind="ExternalInput").ap()
    keysT = nc.dram_tensor("keysT", [D, BK], BF16, kind="ExternalInput").ap()
    Umat = nc.dram_tensor("Umat", [D, D], BF16, kind="ExternalInput").ap()
    Vmat = nc.dram_tensor("Vmat", [D, D], BF16, kind="ExternalInput").ap()
    Wmat = nc.dram_tensor("Wmat", [D, D], BF16, kind="ExternalInput").ap()
    mrow = nc.dram_tensor("mrow", [8, 2 * S], F32, kind="ExternalInput").ap()
    bdm = nc.dram_tensor("bdm", [BL, BK], BF16, kind="ExternalInput").ap()
    hout = nc.dram_tensor("hout", [BK, D], F32, kind="ExternalOutput").ap()

    with tile.TileContext(nc) as tc:
        _emit(nc, tc, table, idx16, keysT, Umat, Vmat, Wmat, mrow, bdm, hout)
    nc.compile()
    return nc


def _emit(nc, tc, table, idx16, keysT, Umat, Vmat, Wmat, mrow, bdm, hout):
    from contextlib import ExitStack

    ctx = ExitStack()
    const = ctx.enter_context(tc.tile_pool(name="const", bufs=1))
    persist = ctx.enter_context(tc.tile_pool(name="persist", bufs=1))
    gpool = ctx.enter_context(tc.tile_pool(name="g", bufs=2))
    work = ctx.enter_context(tc.tile_pool(name="work", bufs=4))
    hpool = ctx.enter_context(tc.tile_pool(name="h", bufs=3))
    # PSUM budget: 8 banks total. psh0+psh1 + {ps, pst, psg, psgb, pss, psi} = 8.
    psH = ctx.enter_context(tc.tile_pool(name="psH", bufs=2, space="PSUM"))
    psS = ctx.enter_context(tc.tile_pool(name="psS", bufs=1, space="PSUM"))

    # ---- constants into SBUF ----
    sb_idx = const.tile([128, NG * TOKG // 16], I16)
    nc.sync.dma_start(out=sb_idx[:], in_=idx16[:])
    kT = [const.tile([128, BK], BF16, tag=f"kT{j}", name=f"kT{j}") for j in range(2)]
    for j in range(2):
        nc.sync.dma_start(out=kT[j][:], in_=keysT[128 * j:128 * (j + 1), :])
    sbU = [const.tile([128, D], BF16, tag=f"sbU{j}", name=f"sbU{j}") for j in range(2)]
    sbV = [const.tile([128, D], BF16, tag=f"sbV{j}", name=f"sbV{j}") for j in range(2)]
    sbW = [const.tile([128, D], BF16, tag=f"sbW{j}", name=f"sbW{j}") for j in range(2)]
    for j in range(2):
        nc.sync.dma_start(out=sbU[j][:], in_=Umat[128 * j:128 * (j + 1), :])
        nc.sync.dma_start(out=sbV[j][:], in_=Vmat[128 * j:128 * (j + 1), :])
        nc.sync.dma_start(out=sbW[j][:], in_=Wmat[128 * j:128 * (j + 1), :])
    sb_m = const.tile([8, 2 * S], F32)
    nc.sync.dma_start(out=sb_m[:], in_=mrow[:])
    sb_bd = const.tile([BL, BK], BF16)
    nc.sync.dma_start(out=sb_bd[:], in_=bdm[:])

    I128 = const.tile([128, 128], BF16)
    make_identity(nc, I128[:])
    ones8 = const.tile([8, 128], BF16)
    nc.vector.memset(ones8[:], 1.0)
    ones128 = const.tile([128, 1], BF16)
    nc.vector.memset(ones128[:], 1.0)
    ones1 = const.tile([1, 128], BF16)
    nc.vector.memset(ones1[:], 1.0)
    epsap = const.tile([1, 1], F32)
    nc.vector.memset(epsap[:], EPS)
    # word-sum reducers: Ablk[i][p, m] = 1 iff m == 4*i + p//32.
    # Slot c contributes sentences 4c+q; accumulating 16 slots with
    # patterns i = c%16 fills a 64-sentence PSUM block (base 0 or 64).
    Ablk = []
    for i in range(16):
        a = const.tile([128, 64], BF16, tag=f"Ablk{i}", name=f"Ablk{i}")
        nc.vector.memset(a[:], 0.0)
        for q in range(4):
            nc.vector.memset(a[32 * q:32 * (q + 1), 4 * i + q:4 * i + q + 1], 1.0)
        Ablk.append(a)

    # ---- persistent intermediates ----
    ET = [persist.tile([128, NG * 128], BF16, tag=f"ET{j}", name=f"ET{j}") for j in range(2)]   # E^T  [d, (g,ds,b)]
    eW = [persist.tile([128, NG * 128], BF16, tag=f"eWt{j}", name=f"eWt{j}") for j in range(2)]   # W^T E^T
    kVT = [persist.tile([128, BK], BF16, tag=f"kVT{j}", name=f"kVT{j}") for j in range(2)]        # V^T keys^T

    # kVT = V^T @ keysT   (out[de, bk] = sum_d V[d,de] keysT[d,bk])
    for m in range(2):
        ps = psS.tile([128, BK], F32, tag="psm0", name="pskv")
        nc.tensor.matmul(ps[:], lhsT=sbV[0][:, 128 * m:128 * (m + 1)], rhs=kT[0][:],
                         start=True, stop=False)
        nc.tensor.matmul(ps[:], lhsT=sbV[1][:, 128 * m:128 * (m + 1)], rhs=kT[1][:],
                         start=False, stop=True)
        nc.vector.tensor_copy(out=kVT[m][:], in_=ps[:])

    # ---- gather groups ----
    for g in range(NG):
        G = gpool.tile([128, L, D], BF16, tag="G")
        nc.gpsimd.dma_gather(
            out_ap=G[:], in_ap=table[:],
            idxs_ap=sb_idx[:, (TOKG // 16) * g:(TOKG // 16) * (g + 1)],
            num_idxs=TOKG, num_idxs_reg=TOKG, elem_size=D, single_packet=False,
        )
        # word-sum: slot c holds words of sentences 4c..4c+3; accumulate
        # 8 slots per 32-aligned PSUM block.
        psE = psS.tile([128, D], F32, tag="psm0", name="psE")
        for c in range(L):
            j, i = c // 16, c % 16
            nc.tensor.matmul(psE[64 * j:64 * (j + 1), :], lhsT=Ablk[i][:],
                             rhs=G[:, c, :], start=(i == 0), stop=(i == 15))
        enc = work.tile([128, D], BF16, tag="enc")
        nc.scalar.copy(out=enc[:], in_=psE[:])
        # transpose -> ET columns for this group
        for j in range(2):
            pt = psS.tile([128, 128], BF16, tag="psm1", name="pt")
            nc.tensor.transpose(pt[:], enc[:, 128 * j:128 * (j + 1)], I128[:])
            nc.vector.tensor_copy(out=ET[j][:, 128 * g:128 * (g + 1)], in_=pt[:])
        # eW = W^T @ ET_g
        for m in range(2):
            pw = psS.tile([128, 128], F32, tag="psm1", name="pw")
            nc.tensor.matmul(pw[:], lhsT=sbW[0][:, 128 * m:128 * (m + 1)],
                             rhs=ET[0][:, 128 * g:128 * (g + 1)], start=True, stop=False)
            nc.tensor.matmul(pw[:], lhsT=sbW[1][:, 128 * m:128 * (m + 1)],
                             rhs=ET[1][:, 128 * g:128 * (g + 1)], start=False, stop=True)
            nc.vector.tensor_copy(out=eW[m][:, 128 * g:128 * (g + 1)], in_=pw[:])

    # ---- scan: two independent batch groups (b 0-7 | b 8-15) pipelined ----
    # Each group owns a 256-wide bk slice and its own PSUM banks, so the two
    # serial dependency chains interleave across engines. Within a group the
    # state h packs both de-tiles side by side ([d0-127 | d128-255] columns)
    # so elementwise V/S ops run full-width [128, 512] in single instructions;
    # the gate/inv broadcasts are duplicated across both column halves.
    HB = BK // 2  # 256
    h = [hpool.tile([128, BK], BF16, tag=f"h{gb}", name=f"h{gb}")
         for gb in range(2)]
    for gb in range(2):
        nc.vector.memset(h[gb][:], 0.0)

    for t in range(S):
        g, ds = t // 8, t % 8
        hn = [None, None]
        for gb in range(2):
            cg = 128 * g + 16 * ds + 8 * gb  # ET/eW cols for this step+group
            bks = slice(HB * gb, HB * (gb + 1))
            hg = h[gb]

            # pshG packs both de tiles: [:, 0:256] = de 0-127, [:, 256:512] = de 128-255
            pshG = psH.tile([128, BK], F32, tag=f"psh{gb}", name=f"psh{gb}")
            for m in range(2):
                msl = slice(HB * m, HB * (m + 1))
                # h-independent terms first: they execute before h_t exists,
                # leaving only the two U^T h matmuls on the critical chain.
                nc.tensor.matmul(pshG[:, msl], lhsT=I128[:], rhs=kVT[m][:, bks],
                                 start=True, stop=False)
                ew_bc = eW[m][:, cg:cg + 8].unsqueeze(2).broadcast_to([128, 8, 32])
                nc.tensor.matmul(pshG[:, msl], lhsT=I128[:], rhs=ew_bc,
                                 start=False, stop=False)
                nc.tensor.matmul(pshG[:, msl], lhsT=sbU[0][:, 128 * m:128 * (m + 1)],
                                 rhs=hg[:, 0:HB], start=False, stop=False)
                nc.tensor.matmul(pshG[:, msl], lhsT=sbU[1][:, 128 * m:128 * (m + 1)],
                                 rhs=hg[:, HB:BK], start=False, stop=True)

            # psMISC: [0:8, 0:256] = gate logits, [0:1, 256:512] = sumsq
            psM = psS.tile([128, BK], F32, tag=f"psm{gb}", name=f"psm{gb}")
            psg = psM[0:8, 0:HB]
            nc.tensor.matmul(psg, lhsT=ET[0][:, cg:cg + 8], rhs=kT[0][:, bks],
                             start=True, stop=False)
            nc.tensor.matmul(psg, lhsT=ET[1][:, cg:cg + 8], rhs=kT[1][:, bks],
                             start=False, stop=False)
            nc.tensor.matmul(psg, lhsT=ET[0][:, cg:cg + 8], rhs=hg[:, 0:HB],
                             start=False, stop=False)
            nc.tensor.matmul(psg, lhsT=ET[1][:, cg:cg + 8], rhs=hg[:, HB:BK],
                             start=False, stop=True)

            # sigmoid = 1/(1+exp(-x)): exp+add on ScalarE, recip on VectorE.
            # No clamp: |logits| < ~30 for this model scale (exp(30) ~ 1e13,
            # far inside reciprocal_approx_fast's safe range).
            eg = work.tile([8, HB], F32, tag=f"eg{gb}", name=f"eg{gb}")
            nc.scalar.activation(eg[:], psg, AF.Exp, scale=-1.0)
            sg = work.tile([8, HB], F32, tag=f"sg{gb}", name=f"sg{gb}")
            nc.vector._custom_dve(_RECIP1P, out=sg[:], in0=eg[:],
                                  s0=float(_R1P_C0), s1=float(_R1P_C1),
                                  imm2=float(_R1P_C2))
            gm = work.tile([8, HB], BF16, tag=f"gm{gb}", name=f"gm{gb}")
            nc.vector.scalar_tensor_tensor(
                out=gm[:], in0=sg[:], scalar=sb_m[0:8, 2 * t + gb:2 * t + gb + 1],
                in1=sb_bd[0:8, 0:HB], op0=OP.mult, op1=OP.mult)
            # gate broadcast duplicated into both column halves
            psBg = psS.tile([128, BK], F32, tag=f"psbg{gb}", name=f"psbg{gb}")
            nc.tensor.matmul(psBg[:, 0:HB], lhsT=ones8[:], rhs=gm[:],
                             start=True, stop=True)
            nc.tensor.matmul(psBg[:, HB:BK], lhsT=ones8[:], rhs=gm[:],
                             start=True, stop=True)

            # full-width elementwise: r = relu(psh); u = r*gate; upd = u + h
            r = work.tile([128, BK], BF16, tag=f"r{gb}", name=f"r{gb}")
            nc.scalar.activation(r[:], pshG[:], AF.Relu)
            u = work.tile([128, BK], BF16, tag=f"u{gb}", name=f"u{gb}")
            nc.vector.tensor_tensor(out=u[:], in0=r[:], in1=psBg[:], op=OP.mult)
            upd = work.tile([128, BK], BF16, tag=f"upd{gb}", name=f"upd{gb}")
            nc.vector.tensor_tensor(out=upd[:], in0=u[:], in1=hg[:], op=OP.add)
            sq = work.tile([128, BK], BF16, tag=f"sq{gb}", name=f"sq{gb}")
            if gb == 0:
                nc.vector.tensor_tensor(out=sq[:], in0=upd[:], in1=upd[:],
                                        op=OP.mult)
            else:
                nc.scalar.activation(sq[:], upd[:], AF.Square)

            pss = psM[0:1, HB:BK]
            nc.tensor.matmul(pss, lhsT=ones128[:], rhs=sq[:, 0:HB],
                             start=True, stop=False)
            nc.tensor.matmul(pss, lhsT=ones128[:], rhs=sq[:, HB:BK],
                             start=False, stop=True)
            lns = work.tile([1, HB], F32, tag=f"lns{gb}", name=f"lns{gb}")
            nc.scalar.activation(lns[:], pss, AF.Ln, bias=epsap[:])
            inv = work.tile([1, HB], BF16, tag=f"inv{gb}", name=f"inv{gb}")
            nc.scalar.activation(inv[:], lns[:], AF.Exp, scale=-0.5)
            # inv broadcast reuses the psM bank (psg/pss are consumed by now;
            # the WAR/WAW edges Tile inserts match the true chain order).
            nc.tensor.matmul(psM[:, 0:HB], lhsT=ones1[:], rhs=inv[:],
                             start=True, stop=True)
            nc.tensor.matmul(psM[:, HB:BK], lhsT=ones1[:], rhs=inv[:],
                             start=True, stop=True)

            hn[gb] = hpool.tile([128, BK], BF16, tag=f"h{gb}", name=f"hn{gb}")
            nc.vector.tensor_tensor(out=hn[gb][:, 0:HB], in0=upd[:, 0:HB],
                                    in1=psM[:, 0:HB], op=OP.mult)
            nc.vector.tensor_tensor(out=hn[gb][:, HB:BK], in0=upd[:, HB:BK],
                                    in1=psM[:, HB:BK], op=OP.mult)
        h = hn

    # ---- output: transpose h^T [256, 512] -> [512, 256] fp32 ----
    for q in range(4):
        gb, half = q // 2, q % 2
        ho = work.tile([128, D], F32, tag="ho")
        for j in range(2):
            pt = psS.tile([128, 128], BF16, tag="psm0", name="ptout")
            nc.tensor.transpose(pt[:], h[gb][:, HB * j + 128 * half:
                                             HB * j + 128 * half + 128], I128[:])
            nc.vector.tensor_copy(out=ho[:, 128 * j:128 * (j + 1)], in_=pt[:])
        nc.sync.dma_start(out=hout[128 * q:128 * (q + 1), :], in_=ho[:])

    ctx.close()


def _prep_core(pr, mask, keys_c, emb):
    """Host-side marshaling for one core's shard."""
    uniq, inv = np.unique(pr, return_inverse=True)
    assert len(uniq) <= TABLE_ROWS
    table = np.zeros((TABLE_ROWS, D), dtype=ml_dtypes.bfloat16)
    table[: len(uniq)] = emb[uniq].astype(ml_dtypes.bfloat16)
    ranks = inv.reshape(BL, S, L).astype(np.int16)

    # token order per group g: i = (ds*16 + b)*32 + w
    idx_groups = []
    for g in range(NG):
        blk = ranks[:, 8 * g:8 * (g + 1), :]          # [b, ds, w]
        lst = blk.transpose(1, 0, 2).reshape(-1)      # [(ds, b, w)] length 4096
        idx_groups.append(np.tile(lst.reshape(TOKG // 16, 16).T, (8, 1)))
    idx16 = np.concatenate(idx_groups, axis=1).astype(np.int16)  # [128, NG*256]

    keysT = np.ascontiguousarray(
        keys_c.reshape(BK, D).T).astype(ml_dtypes.bfloat16)      # [256, 512]
    # mrow2[j, 2t+gb] = mask[8*gb + j, t]  (two pipelined batch groups)
    m = mask.astype(np.float32)                                  # [16, 64]
    mrow2 = np.zeros((8, 2 * S), np.float32)
    for gb in range(2):
        mrow2[:, gb::2] = m[8 * gb:8 * (gb + 1), :]
    return table, idx16, keysT, mrow2


def kernel(prgrph, prgrph_mask, keys, embedding_matrix, U, V, W):
    prgrph = np.asarray(prgrph)
    prgrph_mask = np.asarray(prgrph_mask)
    keys = np.asarray(keys, dtype=np.float32)
    emb = np.asarray(embedding_matrix, dtype=np.float32)
    U = np.asarray(U, dtype=np.float32)
    V = np.asarray(V, dtype=np.float32)
    W = np.asarray(W, dtype=np.float32)

    if "nc" not in _CACHED:
        _CACHED["nc"] = _build_program()
    nc = _CACHED["nc"]

    bd = (np.arange(BL)[:, None] == (np.arange(BK)[None, :] // K)).astype(
        ml_dtypes.bfloat16)
    Ub, Vb, Wb = (x.astype(ml_dtypes.bfloat16) for x in (U, V, W))

    in_maps = []
    for c in range(NC):
        sl = slice(BL * c, BL * (c + 1))
        table, idx16, keysT, mrow = _prep_core(
            prgrph[sl], prgrph_mask[sl, :, 0], keys[sl], emb)
        in_maps.append({
            "table": table, "idx16": idx16, "keysT": keysT,
            "Umat": Ub, "Vmat": Vb, "Wmat": Wb,
            "mrow": mrow, "bdm": bd,
        })

    res = run_bass_kernel_spmd(nc, in_maps, core_ids=list(range(NC)))
    out = np.concatenate(
        [res.results[c]["hout"].reshape(BL, K, D) for c in range(NC)], axis=0)
    return out.astype(np.float32)


# revision 28
# speedup vs baseline: 5441.9926x; 1.0153x over previous
"""Trainium2 Bass kernel for nn_BasicRecurrentEntityEncoder.

Full-input contract: kernel(**inputs) takes the complete (unsharded) numpy
inputs and returns the full [B, K, D] float32 output. Internally the batch
is sharded over 8 NeuronCores (data parallel, no collectives), the embedding
bag-of-words gather runs through dma_gather against a per-core compacted
bf16 table, and the 64-step entity recurrence runs in a transposed
[D, (b,k)] layout with bf16 matmul operands.

Key device-side structure per core (B_local=16, K=32, D=256, S=64):
  - 8 gather groups of 128 sentences (4096 tokens, 1 dma_gather each);
    word-sum via block-ones matmuls into PSUM; TensorE transpose to build
    E^T [256, 1024] incrementally.
  - precompute  kVT = V^T keys^T  and  eW = W^T E^T  once per group.
  - the scan runs as TWO independent batch groups (b 0-7 | b 8-15), each
    with its own PSUM banks, so their serial dependency chains pipeline
    across engines. Per step and group: PSUM accumulates
    U^T h + kVT + eW_bcast via matmuls (identity / stride-0-broadcast
    rhs tricks); gate logits PSUM = E_t^T (h + keys); sigmoid =
    1/(1+exp(-x)) with exp on ScalarE and a one-instruction custom DVE
    op for 1/(1+y); normalization rsqrt = exp(-0.5*ln(ss+eps)) on
    ScalarE. Every ScalarE function lives in the
    natural_log_exp_and_others activation table so no table reloads
    occur (the default greedy chooser is patched out).
  - mask folding: h_new = normalize(h + (m*gate) .* h_tilda) is exact for
    masked rows because h is always 0 or unit-norm.
"""

import sys

if "/opt/trn_rl_repo" not in sys.path:
    sys.path.insert(0, "/opt/trn_rl_repo")

import numpy as np
import ml_dtypes

from concourse import bacc, mybir
import concourse.bass as bass
import concourse.tile as tile
from concourse.bass_utils import run_bass_kernel_spmd
from concourse.masks import make_identity

# Force every ScalarE activation onto the one table set that covers all the
# functions this kernel uses (relu/square/exp/ln/copy/identity). The default
# chooser greedily picks the first set per function (exp -> set 0,
# ln -> set 5), inserting a ~550ns table reload per Ln/Exp pair on the
# critical path. Padding the dict keeps act_func_set_id indices aligned
# with act_info.json while making only the all-covering set usable.
_ONE_SET = "natural_log_exp_and_others"


import concourse.hw_specs as _hw_specs
_ORIG_TABLES = _hw_specs.get_activation_tables


def _patched_tables(module_arch):
    real = _ORIG_TABLES(module_arch)
    names = list(real.keys())
    assert _ONE_SET in names, names
    out = {}
    for n in names:
        if n == _ONE_SET:
            out[n] = real[n]
            break
        out[n] = set()
    return out


def _install_table_patch():
    import functools
    cached = functools.cache(_patched_tables)
    bacc.get_activation_tables = cached
    _hw_specs.get_activation_tables = cached


_install_table_patch()

# Custom DVE op: out ~= 1/(1 + in0) in ONE VectorE instruction (8 ALU
# stages): u = in0+1; seed y0 = bitcast(~bits(u)); t = u*y0 lands in
# [-4.5, -4] for any positive u; quadratic minimax fixup P(t) ~= 1/t gives
# out = y0*P(t) at ~1e-5 relative error. Replaces the separate ScalarE
# "+1" hop feeding reciprocal_approx_fast in the sigmoid.
import concourse.dve_ops as _dve_ops
from concourse.dve_spec import AluOp as _AluOp, Bin as _Bin, Spec as _Spec
from concourse.dve_spec import C0 as _C0, C1 as _C1, C2 as _C2, One as _One
from concourse.dve_spec import Src0 as _Src0, lower as _dve_lower
from concourse.dve_spec import _has_src1 as _dve_has_src1
from concourse.dve_uop import DveOpSpec as _DveOpSpec


def _fit_recip1p_consts():
    t = np.linspace(-4.5, -4.0, 2001)
    c = np.polyfit(t, 1.0 / t, 2)  # [c2, c1, c0]
    return float(c[2]), float(c[1]), float(c[0])


_R1P_C2, _R1P_C1, _R1P_C0 = (lambda c: (c[0], c[1], c[2]))(
    np.polyfit(np.linspace(-4.5, -4.0, 2001),
               1.0 / np.linspace(-4.5, -4.0, 2001), 2))


def _recip1p_ref(in0, in1, c0, c1, c2):
    u = (np.asarray(in0, np.float32) + np.float32(1.0)).astype(np.float32)
    y0 = (~u.view(np.int32)).view(np.float32)
    t = u * y0
    return y0 * (c0 + t * (c1 + c2 * t))


def _make_recip1p():
    u = _Bin(_AluOp.ADD, _Src0, _One)
    y0 = _Bin(_AluOp.BITWISE_NOT, u, u)
    t = u * y0
    spec = _Spec(body=y0 * (_C0 + t * (_C1 + _C2 * t)), reference=_recip1p_ref)
    name = "RECIP1P_APPROX_ANT"
    row = 1 + len(_dve_ops.OPS)
    assert row < 0x20
    shas = {}
    for ver in ("v3", "v4"):
        s = _DveOpSpec(name=name, opcode=row, uops=_dve_lower(spec, ver=ver),
                       rd1_en=_dve_has_src1(spec))
        shas[ver] = s.sha(ver)
    op = _dve_ops.DveOp(name, spec, subdim=False, uops_sha=shas)
    _dve_ops.OPS.append(op)
    _dve_ops._SUB_OPCODE_FOR_NAME[name] = row
    _dve_ops.CUSTOM_DVE_SPECS[name] = spec
    return op


_RECIP1P = _make_recip1p()

F32 = mybir.dt.float32
BF16 = mybir.dt.bfloat16
I16 = mybir.dt.int16
AF = mybir.ActivationFunctionType
OP = mybir.AluOpType

B, S, L, K, D = 128, 64, 32, 32, 256
NC = 8
BL = B // NC              # 16 batch rows per core
BK = BL * K               # 512 = free dim of the state
NG = 8                    # gather groups per core (128 sentences each)
TOKG = 128 * L            # 4096 tokens per group
TABLE_ROWS = 32768        # compacted per-core vocab (unique ids <= 32768)
EPS = 1e-12

_CACHED = {}


def _build_program():
    nc = bacc.Bacc("TRN2", target_bir_lowering=False, debug=False, num_devices=NC)

    table = nc.dram_tensor("table", [TABLE_ROWS, D], BF16, kind="ExternalInput").ap()
    idx16 = nc.dram_tensor("idx16", [128, NG * TOKG // 16], I16, kind="ExternalInput").ap()
    keysT = nc.dram_tensor("keysT", [D, BK], BF16, kind="ExternalInput").ap()
    Umat = nc.dram_tensor("Umat", [D, D], BF16, kind="ExternalInput").ap()
    Vmat = nc.dram_tensor("Vmat", [D, D], BF16, kind="ExternalInput").ap()
    Wmat = nc.dram_tensor("Wmat", [D, D], BF16, kind="ExternalInput").ap()
    mrow = nc.dram_tensor("mrow", [8, 2 * S], F32, kind="ExternalInput").ap()
    bdm = nc.dram_tensor("bdm", [BL, BK], BF16, kind="ExternalInput").ap()
    hout = nc.dram_tensor("hout", [BK, D], F32, kind="ExternalOutput").ap()

    with tile.TileContext(nc) as tc:
        _emit(nc, tc, table, idx16, keysT, Umat, Vmat, Wmat, mrow, bdm, hout)
    nc.compile()
    return nc


def _emit(nc, tc, table, idx16, keysT, Umat, Vmat, Wmat, mrow, bdm, hout):
    from contextlib import ExitStack

    ctx = ExitStack()
    const = ctx.enter_context(tc.tile_pool(name="const", bufs=1))
    persist = ctx.enter_context(tc.tile_pool(name="persist", bufs=1))
    gpool = ctx.enter_context(tc.tile_pool(name="g", bufs=2))
    work = ctx.enter_context(tc.tile_pool(name="work", bufs=4))
    hpool = ctx.enter_context(tc.tile_pool(name="h", bufs=3))
    # PSUM budget: 8 banks total. psh0+psh1 + {ps, pst, psg, psgb, pss, psi} = 8.
    psH = ctx.enter_context(tc.tile_pool(name="psH", bufs=2, space="PSUM"))
    psS = ctx.enter_context(tc.tile_pool(name="psS", bufs=1, space="PSUM"))

    # ---- constants into SBUF ----
    sb_idx = const.tile([128, NG * TOKG // 16], I16)
    nc.sync.dma_start(out=sb_idx[:], in_=idx16[:])
    kT = [const.tile([128, BK], BF16, tag=f"kT{j}", name=f"kT{j}") for j in range(2)]
    for j in range(2):
        nc.sync.dma_start(out=kT[j][:], in_=keysT[128 * j:128 * (j + 1), :])
    sbU = [const.tile([128, D], BF16, tag=f"sbU{j}", name=f"sbU{j}") for j in range(2)]
    sbV = [const.tile([128, D], BF16, tag=f"sbV{j}", name=f"sbV{j}") for j in range(2)]
    sbW = [const.tile([128, D], BF16, tag=f"sbW{j}", name=f"sbW{j}") for j in range(2)]
    for j in range(2):
        nc.sync.dma_start(out=sbU[j][:], in_=Umat[128 * j:128 * (j + 1), :])
        nc.sync.dma_start(out=sbV[j][:], in_=Vmat[128 * j:128 * (j + 1), :])
        nc.sync.dma_start(out=sbW[j][:], in_=Wmat[128 * j:128 * (j + 1), :])
    sb_m = const.tile([8, 2 * S], F32)
    nc.sync.dma_start(out=sb_m[:], in_=mrow[:])
    sb_bd = const.tile([BL, BK], BF16)
    nc.sync.dma_start(out=sb_bd[:], in_=bdm[:])

    I128 = const.tile([128, 128], BF16)
    make_identity(nc, I128[:])
    ones8 = const.tile([8, 128], BF16)
    nc.vector.memset(ones8[:], 1.0)
    ones128 = const.tile([128, 1], BF16)
    nc.vector.memset(ones128[:], 1.0)
    ones1 = const.tile([1, 128], BF16)
    nc.vector.memset(ones1[:], 1.0)
    epsap = const.tile([1, 1], F32)
    nc.vector.memset(epsap[:], EPS)
    # word-sum reducers: Ablk[i][p, m] = 1 iff m == 4*i + p//32.
    # Slot c contributes sentences 4c+q; accumulating 16 slots with
    # patterns i = c%16 fills a 64-sentence PSUM block (base 0 or 64).
    Ablk = []
    for i in range(16):
        a = const.tile([128, 64], BF16, tag=f"Ablk{i}", name=f"Ablk{i}")
        nc.vector.memset(a[:], 0.0)
        for q in range(4):
            nc.vector.memset(a[32 * q:32 * (q + 1), 4 * i + q:4 * i + q + 1], 1.0)
        Ablk.append(a)

    # ---- persistent intermediates ----
    ET = [persist.tile([128, NG * 128], BF16, tag=f"ET{j}", name=f"ET{j}") for j in range(2)]   # E^T  [d, (g,ds,b)]
    eW = [persist.tile([128, NG * 128], BF16, tag=f"eWt{j}", name=f"eWt{j}") for j in range(2)]   # W^T E^T
    kVT = [persist.tile([128, BK], BF16, tag=f"kVT{j}", name=f"kVT{j}") for j in range(2)]        # V^T keys^T

    # kVT = V^T @ keysT   (out[de, bk] = sum_d V[d,de] keysT[d,bk])
    for m in range(2):
        ps = psS.tile([128, BK], F32, tag="psm0", name="pskv")
        nc.tensor.matmul(ps[:], lhsT=sbV[0][:, 128 * m:128 * (m + 1)], rhs=kT[0][:],
                         start=True, stop=False)
        nc.tensor.matmul(ps[:], lhsT=sbV[1][:, 128 * m:128 * (m + 1)], rhs=kT[1][:],
                         start=False, stop=True)
        nc.vector.tensor_copy(out=kVT[m][:], in_=ps[:])

    # ---- gather groups ----
    for g in range(NG):
        G = gpool.tile([128, L, D], BF16, tag="G")
        nc.gpsimd.dma_gather(
            out_ap=G[:], in_ap=table[:],
            idxs_ap=sb_idx[:, (TOKG // 16) * g:(TOKG // 16) * (g + 1)],
            num_idxs=TOKG, num_idxs_reg=TOKG, elem_size=D, single_packet=False,
        )
        # word-sum: slot c holds words of sentences 4c..4c+3; accumulate
        # 8 slots per 32-aligned PSUM block.
        psE = psS.tile([128, D], F32, tag="psm0", name="psE")
        for c in range(L):
            j, i = c // 16, c % 16
            nc.tensor.matmul(psE[64 * j:64 * (j + 1), :], lhsT=Ablk[i][:],
                             rhs=G[:, c, :], start=(i == 0), stop=(i == 15))
        enc = work.tile([128, D], BF16, tag="enc")
        nc.scalar.copy(out=enc[:], in_=psE[:])
        # transpose -> ET columns for this group
        for j in range(2):
            pt = psS.tile([128, 128], BF16, tag="psm1", name="pt")
            nc.tensor.transpose(pt[:], enc[:, 128 * j:128 * (j + 1)], I128[:])
            nc.vector.tensor_copy(out=ET[j][:, 128 * g:128 * (g + 1)], in_=pt[:])
        # eW = W^T @ ET_g
        for m in range(2):
            pw = psS.tile([128, 128], F32, tag="psm1", name="pw")
            nc.tensor.matmul(pw[:], lhsT=sbW[0][:, 128 * m:128 * (m + 1)],
                             rhs=ET[0][:, 128 * g:128 * (g + 1)], start=True, stop=False)
            nc.tensor.matmul(pw[:], lhsT=sbW[1][:, 128 * m:128 * (m + 1)],
                             rhs=ET[1][:, 128 * g:128 * (g + 1)], start=False, stop=True)
            nc.vector.tensor_copy(out=eW[m][:, 128 * g:128 * (g + 1)], in_=pw[:])

    # ---- scan: two independent batch groups (b 0-7 | b 8-15) pipelined ----
    # Each group owns a 256-wide bk slice and its own PSUM banks, so the two
    # serial dependency chains interleave across engines. Within a group the
    # state h packs both de-tiles side by side ([d0-127 | d128-255] columns)
    # so elementwise V/S ops run full-width [128, 512] in single instructions;
    # the gate/inv broadcasts are duplicated across both column halves.
    HB = BK // 2  # 256
    h = [hpool.tile([128, BK], BF16, tag=f"h{gb}", name=f"h{gb}")
         for gb in range(2)]
    for gb in range(2):
        nc.vector.memset(h[gb][:], 0.0)

    for t in range(S):
        g, ds = t // 8, t % 8
        hn = [None, None]
        for gb in range(2):
            cg = 128 * g + 16 * ds + 8 * gb  # ET/eW cols for this step+group
            bks = slice(HB * gb, HB * (gb + 1))
            hg = h[gb]

            # pshG packs both de tiles: [:, 0:256] = de 0-127, [:, 256:512] = de 128-255
            pshG = psH.tile([128, BK], F32, tag=f"psh{gb}", name=f"psh{gb}")
            for m in range(2):
                msl = slice(HB * m, HB * (m + 1))
                # h-independent terms first: they execute before h_t exists,
                # leaving only the two U^T h matmuls on the critical chain.
                nc.tensor.matmul(pshG[:, msl], lhsT=I128[:], rhs=kVT[m][:, bks],
                                 start=True, stop=False)
                ew_bc = eW[m][:, cg:cg + 8].unsqueeze(2).broadcast_to([128, 8, 32])
                nc.tensor.matmul(pshG[:, msl], lhsT=I128[:], rhs=ew_bc,
                                 start=False, stop=False)
                nc.tensor.matmul(pshG[:, msl], lhsT=sbU[0][:, 128 * m:128 * (m + 1)],
                                 rhs=hg[:, 0:HB], start=False, stop=False)
                nc.tensor.matmul(pshG[:, msl], lhsT=sbU[1][:, 128 * m:128 * (m + 1)],
                                 rhs=hg[:, HB:BK], start=False, stop=True)

            # psMISC: [0:8, 0:256] = gate logits, [0:1, 256:512] = sumsq
            psM = psS.tile([128, BK], F32, tag=f"psm{gb}", name=f"psm{gb}")
            psg = psM[0:8, 0:HB]
            nc.tensor.matmul(psg, lhsT=ET[0][:, cg:cg + 8], rhs=kT[0][:, bks],
                             start=True, stop=False)
            nc.tensor.matmul(psg, lhsT=ET[1][:, cg:cg + 8], rhs=kT[1][:, bks],
                             start=False, stop=False)
            nc.tensor.matmul(psg, lhsT=ET[0][:, cg:cg + 8], rhs=hg[:, 0:HB],
                             start=False, stop=False)
            nc.tensor.matmul(psg, lhsT=ET[1][:, cg:cg + 8], rhs=hg[:, HB:BK],
                             start=False, stop=True)

            # sigmoid = 1/(1+exp(-x)): exp+add on ScalarE, recip on VectorE.
            # No clamp: |logits| < ~30 for this model scale (exp(30) ~ 1e13,
            # far inside reciprocal_approx_fast's safe range).
            eg = work.tile([8, HB], F32, tag=f"eg{gb}", name=f"eg{gb}")
            nc.scalar.activation(eg[:], psg, AF.Exp, scale=-1.0)
            sg = work.tile([8, HB], F32, tag=f"sg{gb}", name=f"sg{gb}")
            nc.vector._custom_dve(_RECIP1P, out=sg[:], in0=eg[:],
                                  s0=float(_R1P_C0), s1=float(_R1P_C1),
                                  imm2=float(_R1P_C2))
            gm = work.tile([8, HB], BF16, tag=f"gm{gb}", name=f"gm{gb}")
            nc.vector.scalar_tensor_tensor(
                out=gm[:], in0=sg[:], scalar=sb_m[0:8, 2 * t + gb:2 * t + gb + 1],
                in1=sb_bd[0:8, 0:HB], op0=OP.mult, op1=OP.mult)
            # gate broadcast duplicated into both column halves
            psBg = psS.tile([128, BK], F32, tag=f"psbg{gb}", name=f"psbg{gb}")
            nc.tensor.matmul(psBg[:, 0:HB], lhsT=ones8[:], rhs=gm[:],
                             start=True, stop=True)
            nc.tensor.matmul(psBg[:, HB:BK], lhsT=ones8[:], rhs=gm[:],
                             start=True, stop=True)

            # full-width elementwise: r = relu(psh); u = r*gate; upd = u + h
            r = work.tile([128, BK], BF16, tag=f"r{gb}", name=f"r{gb}")
            nc.scalar.activation(r[:], pshG[:], AF.Relu)
            u = work.tile([128, BK], BF16, tag=f"u{gb}", name=f"u{gb}")
            nc.vector.tensor_tensor(out=u[:], in0=r[:], in1=psBg[:], op=OP.mult)
            upd = work.tile([128, BK], BF16, tag=f"upd{gb}", name=f"upd{gb}")
            nc.vector.tensor_tensor(out=upd[:], in0=u[:], in1=hg[:], op=OP.add)
            sq = work.tile([128, BK], BF16, tag=f"sq{gb}", name=f"sq{gb}")
            if gb == 0:
                nc.vector.tensor_tensor(out=sq[:], in0=upd[:], in1=upd[:],
                                        op=OP.mult)
            else:
                nc.scalar.activation(sq[:], upd[:], AF.Square)

            pss = psM[0:1, HB:BK]
            nc.tensor.matmul(pss, lhsT=ones128[:], rhs=sq[:, 0:HB],
                             start=True, stop=False)
            nc.tensor.matmul(pss, lhsT=ones128[:], rhs=sq[:, HB:BK],
                             start=False, stop=True)
            lns = work.tile([1, HB], F32, tag=f"lns{gb}", name=f"lns{gb}")
            nc.scalar.activation(lns[:], pss, AF.Ln, bias=epsap[:])
            inv = work.tile([1, HB], BF16, tag=f"inv{gb}", name=f"inv{gb}")
            nc.scalar.activation(inv[:], lns[:], AF.Exp, scale=-0.5)
            # inv broadcast reuses the psM bank (psg/pss are consumed by now;
            # the WAR/WAW edges Tile inserts match the true chain order).
            nc.tensor.matmul(psM[:, 0:HB], lhsT=ones1[:], rhs=inv[:],
                             start=True, stop=True)
            nc.tensor.matmul(psM[:, HB:BK], lhsT=ones1[:], rhs=inv[:],
                             start=True, stop=True)

            hn[gb] = hpool.tile([128, BK], BF16, tag=f"h{gb}", name=f"hn{gb}")
            nc.vector.tensor_tensor(out=hn[gb][:, 0:HB], in0=upd[:, 0:HB],
                                    in1=psM[:, 0:HB], op=OP.mult)
            nc.vector.tensor_tensor(out=hn[gb][:, HB:BK], in0=upd[:, HB:BK],
                                    in1=psM[:, HB:BK], op=OP.mult)
        h = hn

    # ---- output: transpose h^T [256, 512] -> [512, 256] fp32 ----
    for q in range(4):
        gb, half = q // 2, q % 2
        ho = work.tile([128, D], F32, tag="ho")
        for j in range(2):
            pt = psS.tile([128, 128], BF16, tag="psm0", name="ptout")
            nc.tensor.transpose(pt[:], h[gb][:, HB * j + 128 * half:
                                             HB * j + 128 * half + 128], I128[:])
            nc.vector.tensor_copy(out=ho[:, 128 * j:128 * (j + 1)], in_=pt[:])
        nc.sync.dma_start(out=hout[128 * q:128 * (q + 1), :], in_=ho[:])

    ctx.close()


def _prep_core(pr, mask, keys_c, emb):
    """Host-side marshaling for one core's shard."""
    uniq, inv = np.unique(pr, return_inverse=True)
    assert len(uniq) <= TABLE_ROWS
    table = np.zeros((TABLE_ROWS, D), dtype=ml_dtypes.bfloat16)
    table[: len(uniq)] = emb[uniq].astype(ml_dtypes.bfloat16)
    ranks = inv.reshape(BL, S, L).astype(np.int16)

    # token order per group g: i = (ds*16 + b)*32 + w
    idx_groups = []
    for g in range(NG):
        blk = ranks[:, 8 * g:8 * (g + 1), :]          # [b, ds, w]
        lst = blk.transpose(1, 0, 2).reshape(-1)      # [(ds, b, w)] length 4096
        idx_groups.append(np.tile(lst.reshape(TOKG // 16, 16).T, (8, 1)))
    idx16 = np.concatenate(idx_groups, axis=1).astype(np.int16)  # [128, NG*256]

    keysT = np.ascontiguousarray(
        keys_c.reshape(BK, D).T).astype(ml_dtypes.bfloat16)      # [256, 512]
    # mrow2[j, 2t+gb] = mask[8*gb + j, t]  (two pipelined batch groups)
    m = mask.astype(np.float32)                                  # [16, 64]
    mrow2 = np.zeros((8, 2 * S), np.float32)
    for gb in range(2):
        mrow2[:, gb::2] = m[8 * gb:8 * (gb + 1), :]
    return table, idx16, keysT, mrow2


def kernel(prgrph, prgrph_mask, keys, embedding_matrix, U, V, W):
    prgrph = np.asarray(prgrph)
    prgrph_mask = np.asarray(prgrph_mask)
    keys = np.asarray(keys, dtype=np.float32)
    emb = np.asarray(embedding_matrix, dtype=np.float32)
    U = np.asarray(U, dtype=np.float32)
    V = np.asarray(V, dtype=np.float32)
    W = np.asarray(W, dtype=np.float32)

    if "nc" not in _CACHED:
        _CACHED["nc"] = _build_program()
    nc = _CACHED["nc"]

    bd = (np.arange(BL)[:, None] == (np.arange(BK)[None, :] // K)).astype(
        ml_dtypes.bfloat16)
    Ub, Vb, Wb = (x.astype(ml_dtypes.bfloat16) for x in (U, V, W))

    in_maps = []
    for c in range(NC):
        sl = slice(BL * c, BL * (c + 1))
        table, idx16, keysT, mrow = _prep_core(
            prgrph[sl], prgrph_mask[sl, :, 0], keys[sl], emb)
        in_maps.append({
            "table": table, "idx16": idx16, "keysT": keysT,
            "Umat": Ub, "Vmat": Vb, "Wmat": Wb,
            "mrow": mrow, "bdm": bd,
        })

    res = run_bass_kernel_spmd(nc, in_maps, core_ids=list(range(NC)))
    out = np.concatenate(
        [res.results[c]["hout"].reshape(BL, K, D) for c in range(NC)], axis=0)
    return out.astype(np.float32)
